# revision 16
# baseline (speedup 1.0000x reference)
"""Trainium2 Bass kernel for nn_AdaptiveCombinatorialComplexLayer.

Math (per batch b):
    adj   = sigmoid(adj_weights) * adj_base          # banded: diagonals {-32,-1,+1,+32}
    xg    = x * sigmoid(node_importance)[None,:,None]
    x_agg = adj @ xg
    v     = x_agg @ V_w.T ; y_pred = x_agg @ sm
    mix   = sigmoid(v @ mix_w.T + mix_b)
    x_proc= mix*v + (1-mix)*y_pred
    out   = LN(x_proc @ Wf[:, :D].T + bf) * gamma + beta

Weight-only folding (host, exact algebra):
    C     = sm @ WfL.T ; Delta = (V_w.T - sm) @ WfL.T    # WfL = Wf[:, :D]
    q     = V_w.T @ mix_w[0]
    BAND[m,n] = sigmoid(adj_weights[n,m]) * adj_base[n,m] * sigmoid(ni)[m]
              (= ADJG^T, the aggregation matrix transposed)

Device pipeline (aggregation FIRST -> one D-wide band matmul, not two):
    uT    = x^T @ BAND            # [feat, node] banded blocks
    aD    = u @ Delta ; aC = u @ C ; aq = u @ q
    mix   = sigmoid(aq + mix_b)
    z     = mix * aD + aC (+ bf)
    out   = LN(z) (* gamma + beta)          # LN stats via bn_stats/bn_aggr

Band structure exploited: for the 32x32 grid, the off-diagonal 128x128 tile
blocks of BAND have nonzeros only in a 32-wide column strip (boundary rows),
so they are packed and matmul'ed as [128, 32] strips.

Sharding: pure data-parallel over batch, 2 batches per core, weights replicated.
"""

import numpy as np

B, N, D, G = 16, 1024, 512, 32
NCORES = 8
BL = B // NCORES          # batches per core
NT = N // 128             # 8 node tiles of 128
KT = D // 128             # 4 feature tiles of 128
LN_EPS = 1e-5
SW = 32                   # off-diagonal strip width

# off-diagonal blocks (j, i) with |j-i| == 1, in pack order
OFF_BLOCKS = [(j, i) for j in range(NT) for i in (j - 1, j + 1) if 0 <= i < NT]
NOFF = len(OFF_BLOCKS)
OFF_IDX = {ji: t for t, ji in enumerate(OFF_BLOCKS)}
BAND_COLS = NT * 128 + NOFF * SW   # diag blocks then off strips

_cache = {}


def _build(has_bf, has_gamma, has_beta):
    from contextlib import ExitStack

    import concourse.bass as bass
    import concourse.tile as tile
    from concourse import bacc, mybir

    f32 = mybir.dt.float32
    bf16 = mybir.dt.bfloat16
    i32 = mybir.dt.int32
    AF = mybir.ActivationFunctionType
    OP = mybir.AluOpType

    nc = bacc.Bacc(
        "TRN2",
        target_bir_lowering=False,
        debug=False,
        num_devices=NCORES,
    )

    xN = nc.dram_tensor("xN", [BL, N, D], bf16, kind="ExternalInput")
    band = nc.dram_tensor("band", [128, BAND_COLS], bf16, kind="ExternalInput")
    cD = nc.dram_tensor("cD", [128, KT * D], bf16, kind="ExternalInput")
    dD = nc.dram_tensor("dD", [128, KT * D], bf16, kind="ExternalInput")
    qD = nc.dram_tensor("qD", [128, KT], bf16, kind="ExternalInput")
    mb = nc.dram_tensor("mb", [128, 1], f32, kind="ExternalInput")
    if has_bf:
        bfb = nc.dram_tensor("bfb", [128, D], f32, kind="ExternalInput")
    if has_gamma:
        gab = nc.dram_tensor("gab", [128, D], f32, kind="ExternalInput")
    if has_beta:
        beb = nc.dram_tensor("beb", [128, D], f32, kind="ExternalInput")
    out = nc.dram_tensor("out", [BL, N, D], bf16, kind="ExternalOutput")

    def diag_ap(j):
        return band_sb[:, 128 * j : 128 * (j + 1)]

    def off_ap(j, i):
        t = OFF_IDX[(j, i)]
        return band_sb[:, NT * 128 + SW * t : NT * 128 + SW * (t + 1)]

    with ExitStack() as ctx:
        tc = ctx.enter_context(tile.TileContext(nc))
        const = ctx.enter_context(tc.tile_pool(name="const", bufs=1))

        # ---- persistent SBUF tensors ----
        mb_sb = const.tile([128, 1], f32)
        magic = const.tile([128, 2], i32)     # 0x5f3759df for NR rsqrt
        nc.vector.memset(magic[:], 0x5F3759DF)
        junk = const.tile([128, D], bf16)     # PE p-state warmup operand
        nc.vector.memset(junk[:], 0.0)
        band_sb = const.tile([128, BAND_COLS], bf16)
        c_bf = const.tile([128, KT * D], bf16)
        d_bf = const.tile([128, KT * D], bf16)
        q_bf = const.tile([128, KT], bf16)
        if has_bf:
            bf_sb = const.tile([128, D], f32)
            nc.sync.dma_start(bf_sb[:], bfb[:])
        if has_gamma:
            ga_sb = const.tile([128, D], f32)
            nc.sync.dma_start(ga_sb[:], gab[:])
        if has_beta:
            be_sb = const.tile([128, D], f32)
            nc.sync.dma_start(be_sb[:], beb[:])

        xpool = ctx.enter_context(tc.tile_pool(name="xpool", bufs=BL))
        upool = ctx.enter_context(tc.tile_pool(name="upool", bufs=BL))
        xsb = []
        for b in range(BL):
            xsb.append(xpool.tile([128, NT * D], bf16, tag=f"x{b}", name=f"x{b}"))
        usb = [upool.tile([128, NT * D], bf16, tag=f"u{b}", name=f"u{b}")
               for b in range(BL)]

        def load_x(b, jlo, jhi):
            nc.sync.dma_start(
                xsb[b][:, D * jlo : D * jhi].rearrange("p (j d) -> p j d", d=D),
                xN[b, 128 * jlo : 128 * jhi].rearrange("(j p) d -> p j d", p=128),
            )

        # ---- DMA issue order == serial transfer order on the DMA pipe:
        # gate b0 aggregation first, then the weights for the projections,
        # then the rest of x.
        nc.sync.dma_start(band_sb[:], band[:])
        load_x(0, 0, 4)
        nc.sync.dma_start(
            d_bf[:].rearrange("p (k c) -> p k c", k=KT),
            dD[:].rearrange("p (k c) -> p k c", k=KT),
        )
        nc.sync.dma_start(q_bf[:], qD[:])
        nc.sync.dma_start(mb_sb[:], mb[:])
        nc.sync.dma_start(
            c_bf[:].rearrange("p (k c) -> p k c", k=KT),
            cD[:].rearrange("p (k c) -> p k c", k=KT),
        )
        load_x(0, 4, 8)
        load_x(1, 0, 4)
        load_x(1, 4, 8)

        # ---- PSUM pools: 8 banks ----
        psU = ctx.enter_context(tc.tile_pool(name="psU", bufs=2, space="PSUM"))
        psA = ctx.enter_context(tc.tile_pool(name="psA", bufs=2, space="PSUM"))
        psB = ctx.enter_context(tc.tile_pool(name="psB", bufs=2, space="PSUM"))
        psS = ctx.enter_context(tc.tile_pool(name="psS", bufs=2, space="PSUM"))

        epi = ctx.enter_context(tc.tile_pool(name="epi", bufs=4))
        zpool = ctx.enter_context(tc.tile_pool(name="zpool", bufs=3))
        opool = ctx.enter_context(tc.tile_pool(name="opool", bufs=3))

        def emit_warm(n):
            # keep the PE p-state ramp hot across known DMA-pacing stalls;
            # writes are never read (recycled tags)
            for _ in range(n):
                pj = psA.tile([128, D], f32, tag="bigA", name="pj")
                nc.tensor.matmul(pj[:], junk[:, :128], junk[:], start=True, stop=True)

        def emit_agg_tile(b, i):
            """uT tile i of batch b -> usb[b][:, 512i + 128k] (bf16).

            Off-diagonal neighbor blocks touch only a 32-col strip of the
            output: left neighbor -> cols [0,32), right -> cols [96,128)."""
            pu = psU.tile([128, D], f32, tag="u")
            xl = xsb[b]
            for k in range(KT):
                ks = slice(128 * k, 128 * (k + 1))
                lhs_i = xl[:, D * i + 128 * k : D * i + 128 * (k + 1)]
                base = 128 * k
                # segments of the 128 output cols: (lo, hi, with_off, j_off)
                segs = []
                if i > 0:
                    segs.append((0, SW, True, i - 1))
                    segs.append((SW, 128 if i == NT - 1 else 128 - SW, False, 0))
                else:
                    segs.append((0, 128 - SW, False, 0))
                if i < NT - 1:
                    segs.append((128 - SW, 128, True, i + 1))
                for lo, hi, with_off, joff in segs:
                    osl = pu[:, base + lo : base + hi]
                    nc.tensor.matmul(
                        osl, lhs_i, diag_ap(i)[:, lo:hi],
                        start=True, stop=not with_off,
                    )
                    if with_off:
                        lhs_o = xl[:, D * joff + 128 * k : D * joff + 128 * (k + 1)]
                        nc.tensor.matmul(
                            osl, lhs_o, off_ap(joff, i), start=False, stop=True
                        )
            if i % 2 == 0:
                nc.scalar.activation(usb[b][:, D * i : D * (i + 1)], pu[:], AF.Copy)
            else:
                nc.vector.tensor_copy(usb[b][:, D * i : D * (i + 1)], pu[:])

        pending_ot = []

        def flush_ot():
            # deferred one tile so the ACT queue never head-blocks on the
            # (late-ready) scale-shift while the next tile's mix is ready
            while pending_ot:
                b, i, z, y, nmr = pending_ot.pop(0)
                ot = opool.tile([128, D], bf16, tag="ot", name="ot")
                nc.scalar.activation(
                    ot[:], z[:], AF.Identity, bias=nmr[:], scale=y[:]
                )
                if has_gamma:
                    nc.vector.tensor_tensor(ot[:], ot[:], ga_sb[:], OP.mult)
                if has_beta:
                    nc.vector.tensor_tensor(ot[:], ot[:], be_sb[:], OP.add)
                nc.sync.dma_start(out[b, 128 * i : 128 * (i + 1), :], ot[:])

        def emit_proj_tile(b, i, fast_tail=False):
            """Project uT tile i through q/C/Delta (q first: its stop releases
            the mix sigmoid early), then the fused epilogue: mix-combine,
            bn LN stats, NR rsqrt, deferred scale-shift + store.

            fast_tail: split the C/Delta projections and the epilogue into
            column halves so the first half's DVE chain overlaps the second
            half's matmuls (used for the last tiles, where no later PE work
            hides the epilogue)."""
            pa_d = psA.tile([128, D], f32, tag="bigA")
            pa_c = psB.tile([128, D], f32, tag="bigB")
            pa_q = psS.tile([128, 1], f32, tag="sm")
            for k in range(KT):
                lhsT = usb[b][:, D * i + 128 * k : D * i + 128 * (k + 1)]
                nc.tensor.matmul(
                    pa_q[:], lhsT, q_bf[:, k : k + 1],
                    start=k == 0, stop=k == KT - 1,
                )
            halves = (
                [(0, D // 2), (D // 2, D)] if fast_tail else [(0, D)]
            )
            for lo, hi in halves:
                for k in range(KT):
                    lhsT = usb[b][:, D * i + 128 * k : D * i + 128 * (k + 1)]
                    nc.tensor.matmul(
                        pa_c[:, lo:hi], lhsT, c_bf[:, D * k + lo : D * k + hi],
                        start=k == 0, stop=k == KT - 1,
                    )
            mix = epi.tile([128, 1], f32, tag="mix")
            nc.scalar.activation(
                mix[:], pa_q[:], AF.Sigmoid, bias=mb_sb[:], scale=1.0
            )
            # HW: only one non-scalar PSUM operand per instruction
            csb = epi.tile([128, D], f32, tag="csb")
            nc.scalar.activation(csb[:], pa_c[:], AF.Copy)
            flush_ot()
            z = zpool.tile([128, D], f32, tag="z")
            s6 = epi.tile([128, 6 * len(halves)], f32, tag="s6")
            for h, (lo, hi) in enumerate(halves):
                for k in range(KT):
                    lhsT = usb[b][:, D * i + 128 * k : D * i + 128 * (k + 1)]
                    nc.tensor.matmul(
                        pa_d[:, lo:hi], lhsT, d_bf[:, D * k + lo : D * k + hi],
                        start=k == 0, stop=k == KT - 1,
                    )
                nc.vector.scalar_tensor_tensor(
                    z[:, lo:hi], pa_d[:, lo:hi], mix[:], csb[:, lo:hi],
                    OP.mult, OP.add,
                )
                if has_bf:
                    nc.vector.tensor_tensor(
                        z[:, lo:hi], z[:, lo:hi], bf_sb[:, lo:hi], OP.add
                    )
                nc.vector.bn_stats(s6[:, 6 * h : 6 * (h + 1)], z[:, lo:hi])
            s2 = epi.tile([128, 2], f32, tag="s2")
            nc.vector.bn_aggr(s2[:], s6[:])
            # rstd = NR rsqrt(var + eps); nmr = -mean * rstd
            eng = nc.vector
            va = epi.tile([128, 1], f32, tag="va")
            eng.tensor_scalar(va[:], s2[:, 1:2], LN_EPS, None, OP.add)
            ih = epi.tile([128, 1], i32, tag="ih")
            eng.tensor_scalar(ih[:], va[:].bitcast(i32), 1, None, OP.arith_shift_right)
            y = epi.tile([128, 1], f32, tag="y")
            eng.scalar_tensor_tensor(
                y[:].bitcast(i32), magic[:, :1], 0, ih[:], OP.bypass, OP.subtract
            )
            t1 = epi.tile([128, 1], f32, tag="t1")
            eng.tensor_tensor(t1[:], y[:], y[:], OP.mult)
            eng.tensor_tensor(t1[:], t1[:], va[:], OP.mult)
            eng.tensor_scalar(t1[:], t1[:], -0.5, 1.5, OP.mult, OP.add)
            eng.tensor_tensor(y[:], y[:], t1[:], OP.mult)
            nmr = epi.tile([128, 1], f32, tag="nmr")
            eng.tensor_scalar(nmr[:], s2[:, 0:1], y[:], -1.0, OP.mult, OP.mult)
            if fast_tail:
                # scale-shift halves in parallel on ACT + DVE, store at once
                ot = opool.tile([128, D], bf16, tag="ot", name="ot")
                nc.scalar.activation(
                    ot[:, : D // 2], z[:, : D // 2], AF.Identity,
                    bias=nmr[:], scale=y[:],
                )
                nc.vector.tensor_scalar(
                    ot[:, D // 2 :], z[:, D // 2 :], y[:], nmr[:],
                    OP.mult, OP.add,
                )
                if has_gamma:
                    nc.vector.tensor_tensor(ot[:], ot[:], ga_sb[:], OP.mult)
                if has_beta:
                    nc.vector.tensor_tensor(ot[:], ot[:], be_sb[:], OP.add)
                nc.sync.dma_start(out[b, 128 * i : 128 * (i + 1), :], ot[:])
            else:
                pending_ot.append((b, i, z, y, nmr))

        # ---- schedule ----
        emit_warm(9)
        for i in range(4):
            emit_agg_tile(0, i)
        emit_warm(3)
        emit_proj_tile(0, 0)
        emit_proj_tile(0, 1)
        for i in range(4, NT):
            emit_agg_tile(0, i)
            emit_proj_tile(0, i - 2)
        emit_proj_tile(0, NT - 2)
        emit_proj_tile(0, NT - 1)
        for i in range(NT):
            emit_agg_tile(1, i)
            if i >= 2:
                emit_proj_tile(1, i - 2)
        emit_proj_tile(1, NT - 2, fast_tail=True)
        emit_proj_tile(1, NT - 1, fast_tail=True)

    nc.compile()
    return nc


def _get_nc(has_bf, has_gamma, has_beta):
    key = (has_bf, has_gamma, has_beta)
    if key not in _cache:
        _cache[key] = _build(*key)
    return _cache[key]


def _pack_band(band_mat):
    """band_mat: (N, N) ADJG^T; pack 8 diag 128-blocks then the 14 off-diag
    32-col strips (left-neighbor strip = first 32 cols, right = last 32)."""
    outp = np.zeros((128, BAND_COLS), np.float32)
    for j in range(NT):
        outp[:, 128 * j : 128 * (j + 1)] = band_mat[
            128 * j : 128 * (j + 1), 128 * j : 128 * (j + 1)
        ]
    for t, (j, i) in enumerate(OFF_BLOCKS):
        blk = band_mat[128 * j : 128 * (j + 1), 128 * i : 128 * (i + 1)]
        strip = blk[:, :SW] if i > j else blk[:, 128 - SW :]
        # verify nothing outside the strip (grid-band structure)
        outp[:, NT * 128 + SW * t : NT * 128 + SW * (t + 1)] = strip
    return outp


def _pack_rows(mat):
    """mat: (D, D') -> [128, KT*D'] with row-tile k at cols [D'*k, D'*(k+1))."""
    Dp = mat.shape[1]
    return np.ascontiguousarray(
        mat.reshape(KT, 128, Dp).transpose(1, 0, 2).reshape(128, KT * Dp)
    )


def prepare_shared(adj_weights, adj_base, node_importance, V_w, semantic_memory,
                   mix_w, mix_b, Wf, bf, gamma, beta):
    """Host-side weight folding -> shared (per-core replicated) device inputs."""
    import ml_dtypes

    bfl = ml_dtypes.bfloat16
    g = 1.0 / (1.0 + np.exp(-node_importance.astype(np.float64)))
    sig = 1.0 / (1.0 + np.exp(-adj_weights.T.astype(np.float64)))
    band_mat = (sig * adj_base.T.astype(np.float64) * g[:, None]).astype(np.float32)
    band = _pack_band(band_mat).astype(bfl)

    WfL_T = Wf[:, :D].T.astype(np.float32)           # (D, D): WfL_T[k, h] = Wf[h, k]
    sm = semantic_memory.astype(np.float32)
    C = sm @ WfL_T                                    # (D, D)
    Delta = (V_w.astype(np.float32).T - sm) @ WfL_T
    q = V_w.astype(np.float32).T @ mix_w.reshape(-1).astype(np.float32)  # (D,)

    shared = {
        "band": band,
        "cD": _pack_rows(C).astype(bfl),
        "dD": _pack_rows(Delta).astype(bfl),
        "qD": np.ascontiguousarray(q.reshape(KT, 128).T).astype(bfl),
        "mb": np.full((128, 1), float(np.asarray(mix_b).reshape(-1)[0]), np.float32),
    }
    has_bf = bool(np.any(bf != 0.0))
    has_gamma = bool(np.any(gamma != 1.0))
    has_beta = bool(np.any(beta != 0.0))
    if has_bf:
        shared["bfb"] = np.ascontiguousarray(np.tile(bf.reshape(1, D), (128, 1)))
    if has_gamma:
        shared["gab"] = np.ascontiguousarray(np.tile(gamma.reshape(1, D), (128, 1)))
    if has_beta:
        shared["beb"] = np.ascontiguousarray(np.tile(beta.reshape(1, D), (128, 1)))
    return shared, (has_bf, has_gamma, has_beta)


def kernel(
    x,
    adj_weights,
    adj_base,
    node_importance,
    V_w,
    semantic_memory,
    mix_w,
    mix_b,
    Wf,
    bf,
    gamma,
    beta,
):
    from concourse.bass_utils import run_bass_kernel_spmd

    import ml_dtypes

    bfl = ml_dtypes.bfloat16

    x = np.asarray(x, np.float32)
    shared, variant = prepare_shared(
        np.asarray(adj_weights, np.float32),
        np.asarray(adj_base, np.float32),
        np.asarray(node_importance, np.float32),
        np.asarray(V_w, np.float32),
        np.asarray(semantic_memory, np.float32),
        np.asarray(mix_w, np.float32),
        np.asarray(mix_b, np.float32),
        np.asarray(Wf, np.float32),
        np.asarray(bf, np.float32),
        np.asarray(gamma, np.float32),
        np.asarray(beta, np.float32),
    )
    nc = _get_nc(*variant)

    in_maps = []
    for c in range(NCORES):
        m = dict(shared)
        m["xN"] = np.ascontiguousarray(x[BL * c : BL * (c + 1)]).astype(bfl)
        in_maps.append(m)

    res = run_bass_kernel_spmd(nc, in_maps, core_ids=list(range(NCORES)))
    return np.concatenate(
        [res.results[c]["out"].astype(np.float32) for c in range(NCORES)], axis=0
    )


# revision 17
# speedup vs baseline: 1.0309x; 1.0309x over previous
"""Trainium2 Bass kernel for nn_AdaptiveCombinatorialComplexLayer.

Math (per batch b):
    adj   = sigmoid(adj_weights) * adj_base          # banded: diagonals {-32,-1,+1,+32}
    xg    = x * sigmoid(node_importance)[None,:,None]
    x_agg = adj @ xg
    v     = x_agg @ V_w.T ; y_pred = x_agg @ sm
    mix   = sigmoid(v @ mix_w.T + mix_b)
    x_proc= mix*v + (1-mix)*y_pred
    out   = LN(x_proc @ Wf[:, :D].T + bf) * gamma + beta

Weight-only folding (host, exact algebra):
    C     = sm @ WfL.T ; Delta = (V_w.T - sm) @ WfL.T    # WfL = Wf[:, :D]
    q     = V_w.T @ mix_w[0]
    BAND[m,n] = sigmoid(adj_weights[n,m]) * adj_base[n,m] * sigmoid(ni)[m]
              (= ADJG^T, the aggregation matrix transposed)

Device pipeline (aggregation FIRST -> one D-wide band matmul, not two):
    uT    = x^T @ BAND            # [feat, node] banded blocks
    aD    = u @ Delta ; aC = u @ C ; aq = u @ q
    mix   = sigmoid(aq + mix_b)
    z     = mix * aD + aC (+ bf)
    out   = LN(z) (* gamma + beta)          # LN stats via bn_stats/bn_aggr

Band structure exploited: for the 32x32 grid, the off-diagonal 128x128 tile
blocks of BAND have nonzeros only in a 32-wide column strip (boundary rows),
so they are packed and matmul'ed as [128, 32] strips.

Sharding: pure data-parallel over batch, 2 batches per core, weights replicated.
"""

import numpy as np

B, N, D, G = 16, 1024, 512, 32
NCORES = 8
BL = B // NCORES          # batches per core
NT = N // 128             # 8 node tiles of 128
KT = D // 128             # 4 feature tiles of 128
LN_EPS = 1e-5
SW = 32                   # off-diagonal strip width

# off-diagonal blocks (j, i) with |j-i| == 1, in pack order
OFF_BLOCKS = [(j, i) for j in range(NT) for i in (j - 1, j + 1) if 0 <= i < NT]
NOFF = len(OFF_BLOCKS)
OFF_IDX = {ji: t for t, ji in enumerate(OFF_BLOCKS)}
BAND_COLS = NT * 128 + NOFF * SW   # diag blocks then off strips

_cache = {}


def _build(has_bf, has_gamma, has_beta):
    from contextlib import ExitStack

    import concourse.bass as bass
    import concourse.tile as tile
    from concourse import bacc, mybir

    f32 = mybir.dt.float32
    bf16 = mybir.dt.bfloat16
    i32 = mybir.dt.int32
    AF = mybir.ActivationFunctionType
    OP = mybir.AluOpType

    nc = bacc.Bacc(
        "TRN2",
        target_bir_lowering=False,
        debug=False,
        num_devices=NCORES,
    )

    xN = nc.dram_tensor("xN", [BL, N, D], bf16, kind="ExternalInput")
    band = nc.dram_tensor("band", [128, BAND_COLS], bf16, kind="ExternalInput")
    cD = nc.dram_tensor("cD", [128, KT * D], bf16, kind="ExternalInput")
    dD = nc.dram_tensor("dD", [128, KT * D], bf16, kind="ExternalInput")
    qD = nc.dram_tensor("qD", [128, KT], bf16, kind="ExternalInput")
    mb = nc.dram_tensor("mb", [128, 1], f32, kind="ExternalInput")
    if has_bf:
        bfb = nc.dram_tensor("bfb", [128, D], f32, kind="ExternalInput")
    if has_gamma:
        gab = nc.dram_tensor("gab", [128, D], f32, kind="ExternalInput")
    if has_beta:
        beb = nc.dram_tensor("beb", [128, D], f32, kind="ExternalInput")
    out = nc.dram_tensor("out", [BL, N, D], bf16, kind="ExternalOutput")

    def diag_ap(j):
        return band_sb[:, 128 * j : 128 * (j + 1)]

    def off_ap(j, i):
        t = OFF_IDX[(j, i)]
        return band_sb[:, NT * 128 + SW * t : NT * 128 + SW * (t + 1)]

    with ExitStack() as ctx:
        tc = ctx.enter_context(tile.TileContext(nc))
        const = ctx.enter_context(tc.tile_pool(name="const", bufs=1))

        # ---- persistent SBUF tensors ----
        mb_sb = const.tile([128, 1], f32)
        magic = const.tile([128, 2], i32)     # 0x5f3759df for NR rsqrt
        nc.vector.memset(magic[:], 0x5F3759DF)
        junk = const.tile([128, D], bf16)     # PE p-state warmup operand
        nc.vector.memset(junk[:], 0.0)
        band_sb = const.tile([128, BAND_COLS], bf16)
        c_bf = const.tile([128, KT * D], bf16)
        d_bf = const.tile([128, KT * D], bf16)
        q_bf = const.tile([128, KT], bf16)
        if has_bf:
            bf_sb = const.tile([128, D], f32)
            nc.sync.dma_start(bf_sb[:], bfb[:])
        if has_gamma:
            ga_sb = const.tile([128, D], f32)
            nc.sync.dma_start(ga_sb[:], gab[:])
        if has_beta:
            be_sb = const.tile([128, D], f32)
            nc.sync.dma_start(be_sb[:], beb[:])

        xpool = ctx.enter_context(tc.tile_pool(name="xpool", bufs=BL))
        upool = ctx.enter_context(tc.tile_pool(name="upool", bufs=BL))
        xsb = []
        for b in range(BL):
            xsb.append(xpool.tile([128, NT * D], bf16, tag=f"x{b}", name=f"x{b}"))
        usb = [upool.tile([128, NT * D], bf16, tag=f"u{b}", name=f"u{b}")
               for b in range(BL)]

        def load_x(b, jlo, jhi):
            nc.sync.dma_start(
                xsb[b][:, D * jlo : D * jhi].rearrange("p (j d) -> p j d", d=D),
                xN[b, 128 * jlo : 128 * jhi].rearrange("(j p) d -> p j d", p=128),
            )

        # ---- DMA issue order == serial transfer order on the DMA pipe:
        # gate b0 aggregation first, then the weights for the projections,
        # then the rest of x.
        nc.sync.dma_start(band_sb[:], band[:])
        load_x(0, 0, 4)
        nc.sync.dma_start(
            d_bf[:].rearrange("p (k c) -> p k c", k=KT),
            dD[:].rearrange("p (k c) -> p k c", k=KT),
        )
        nc.sync.dma_start(q_bf[:], qD[:])
        nc.sync.dma_start(mb_sb[:], mb[:])
        nc.sync.dma_start(
            c_bf[:].rearrange("p (k c) -> p k c", k=KT),
            cD[:].rearrange("p (k c) -> p k c", k=KT),
        )
        load_x(0, 4, 8)
        load_x(1, 0, 4)
        load_x(1, 4, 8)

        # ---- PSUM pools: 8 banks ----
        psU = ctx.enter_context(tc.tile_pool(name="psU", bufs=2, space="PSUM"))
        psA = ctx.enter_context(tc.tile_pool(name="psA", bufs=2, space="PSUM"))
        psB = ctx.enter_context(tc.tile_pool(name="psB", bufs=2, space="PSUM"))
        psS = ctx.enter_context(tc.tile_pool(name="psS", bufs=2, space="PSUM"))

        epi = ctx.enter_context(tc.tile_pool(name="epi", bufs=4))
        zpool = ctx.enter_context(tc.tile_pool(name="zpool", bufs=3))
        opool = ctx.enter_context(tc.tile_pool(name="opool", bufs=3))

        def emit_warm(n):
            # keep the PE p-state ramp hot across known DMA-pacing stalls;
            # writes are never read (recycled tags)
            for _ in range(n):
                pj = psA.tile([128, D], f32, tag="bigA", name="pj")
                nc.tensor.matmul(pj[:], junk[:, :128], junk[:], start=True, stop=True)

        def emit_agg_tile(b, i):
            """uT tile i of batch b -> usb[b][:, 512i + 128k] (bf16).

            Off-diagonal neighbor blocks touch only a 32-col strip of the
            output: left neighbor -> cols [0,32), right -> cols [96,128)."""
            pu = psU.tile([128, D], f32, tag="u")
            xl = xsb[b]
            for k in range(KT):
                ks = slice(128 * k, 128 * (k + 1))
                lhs_i = xl[:, D * i + 128 * k : D * i + 128 * (k + 1)]
                base = 128 * k
                # segments of the 128 output cols: (lo, hi, with_off, j_off)
                segs = []
                if i > 0:
                    segs.append((0, SW, True, i - 1))
                    segs.append((SW, 128 if i == NT - 1 else 128 - SW, False, 0))
                else:
                    segs.append((0, 128 - SW, False, 0))
                if i < NT - 1:
                    segs.append((128 - SW, 128, True, i + 1))
                for lo, hi, with_off, joff in segs:
                    osl = pu[:, base + lo : base + hi]
                    nc.tensor.matmul(
                        osl, lhs_i, diag_ap(i)[:, lo:hi],
                        start=True, stop=not with_off,
                    )
                    if with_off:
                        lhs_o = xl[:, D * joff + 128 * k : D * joff + 128 * (k + 1)]
                        nc.tensor.matmul(
                            osl, lhs_o, off_ap(joff, i), start=False, stop=True
                        )
            nc.scalar.activation(usb[b][:, D * i : D * (i + 1)], pu[:], AF.Copy)

        pending_ot = []

        def flush_ot():
            # deferred one tile so the ACT queue never head-blocks on the
            # (late-ready) scale-shift while the next tile's mix is ready
            while pending_ot:
                b, i, z, y, nmr = pending_ot.pop(0)
                ot = opool.tile([128, D], bf16, tag="ot", name="ot")
                nc.scalar.activation(
                    ot[:], z[:], AF.Identity, bias=nmr[:], scale=y[:]
                )
                if has_gamma:
                    nc.vector.tensor_tensor(ot[:], ot[:], ga_sb[:], OP.mult)
                if has_beta:
                    nc.vector.tensor_tensor(ot[:], ot[:], be_sb[:], OP.add)
                nc.sync.dma_start(out[b, 128 * i : 128 * (i + 1), :], ot[:])

        def emit_proj_tile(b, i, fast_tail=False):
            """Project uT tile i through q/C/Delta (q first: its stop releases
            the mix sigmoid early), then the fused epilogue: mix-combine,
            bn LN stats, NR rsqrt, deferred scale-shift + store.

            fast_tail: split the C/Delta projections and the epilogue into
            column halves so the first half's DVE chain overlaps the second
            half's matmuls (used for the last tiles, where no later PE work
            hides the epilogue)."""
            pa_d = psA.tile([128, D], f32, tag="bigA")
            pa_c = psB.tile([128, D], f32, tag="bigB")
            pa_q = psS.tile([128, 1], f32, tag="sm")
            for k in range(KT):
                lhsT = usb[b][:, D * i + 128 * k : D * i + 128 * (k + 1)]
                nc.tensor.matmul(
                    pa_q[:], lhsT, q_bf[:, k : k + 1],
                    start=k == 0, stop=k == KT - 1,
                )
            halves = (
                [(0, D // 2), (D // 2, D)] if fast_tail else [(0, D)]
            )
            for lo, hi in halves:
                for k in range(KT):
                    lhsT = usb[b][:, D * i + 128 * k : D * i + 128 * (k + 1)]
                    nc.tensor.matmul(
                        pa_c[:, lo:hi], lhsT, c_bf[:, D * k + lo : D * k + hi],
                        start=k == 0, stop=k == KT - 1,
                    )
            mix = epi.tile([128, 1], f32, tag="mix")
            nc.scalar.activation(
                mix[:], pa_q[:], AF.Sigmoid, bias=mb_sb[:], scale=1.0
            )
            # HW: only one non-scalar PSUM operand per instruction
            csb = epi.tile([128, D], f32, tag="csb")
            nc.scalar.activation(csb[:], pa_c[:], AF.Copy)
            flush_ot()
            z = zpool.tile([128, D], f32, tag="z")
            s6 = epi.tile([128, 6 * len(halves)], f32, tag="s6")
            for h, (lo, hi) in enumerate(halves):
                for k in range(KT):
                    lhsT = usb[b][:, D * i + 128 * k : D * i + 128 * (k + 1)]
                    nc.tensor.matmul(
                        pa_d[:, lo:hi], lhsT, d_bf[:, D * k + lo : D * k + hi],
                        start=k == 0, stop=k == KT - 1,
                    )
                nc.vector.scalar_tensor_tensor(
                    z[:, lo:hi], pa_d[:, lo:hi], mix[:], csb[:, lo:hi],
                    OP.mult, OP.add,
                )
                if has_bf:
                    nc.vector.tensor_tensor(
                        z[:, lo:hi], z[:, lo:hi], bf_sb[:, lo:hi], OP.add
                    )
                nc.vector.bn_stats(s6[:, 6 * h : 6 * (h + 1)], z[:, lo:hi])
            s2 = epi.tile([128, 2], f32, tag="s2")
            nc.vector.bn_aggr(s2[:], s6[:])
            # rstd = NR rsqrt(var + eps); nmr = -mean * rstd
            eng = nc.vector
            va = epi.tile([128, 1], f32, tag="va")
            eng.tensor_scalar(va[:], s2[:, 1:2], LN_EPS, None, OP.add)
            ih = epi.tile([128, 1], i32, tag="ih")
            eng.tensor_scalar(ih[:], va[:].bitcast(i32), 1, None, OP.arith_shift_right)
            y = epi.tile([128, 1], f32, tag="y")
            eng.scalar_tensor_tensor(
                y[:].bitcast(i32), magic[:, :1], 0, ih[:], OP.bypass, OP.subtract
            )
            t1 = epi.tile([128, 1], f32, tag="t1")
            eng.tensor_tensor(t1[:], y[:], y[:], OP.mult)
            eng.tensor_tensor(t1[:], t1[:], va[:], OP.mult)
            eng.tensor_scalar(t1[:], t1[:], -0.5, 1.5, OP.mult, OP.add)
            eng.tensor_tensor(y[:], y[:], t1[:], OP.mult)
            nmr = epi.tile([128, 1], f32, tag="nmr")
            eng.tensor_scalar(nmr[:], s2[:, 0:1], y[:], -1.0, OP.mult, OP.mult)
            if fast_tail:
                # scale-shift halves in parallel on ACT + DVE, store at once
                ot = opool.tile([128, D], bf16, tag="ot", name="ot")
                nc.scalar.activation(
                    ot[:, : D // 2], z[:, : D // 2], AF.Identity,
                    bias=nmr[:], scale=y[:],
                )
                nc.vector.tensor_scalar(
                    ot[:, D // 2 :], z[:, D // 2 :], y[:], nmr[:],
                    OP.mult, OP.add,
                )
                if has_gamma:
                    nc.vector.tensor_tensor(ot[:], ot[:], ga_sb[:], OP.mult)
                if has_beta:
                    nc.vector.tensor_tensor(ot[:], ot[:], be_sb[:], OP.add)
                nc.sync.dma_start(out[b, 128 * i : 128 * (i + 1), :], ot[:])
            else:
                pending_ot.append((b, i, z, y, nmr))

        # ---- schedule ----
        emit_warm(9)
        for i in range(4):
            emit_agg_tile(0, i)
        emit_warm(3)
        emit_proj_tile(0, 0)
        emit_proj_tile(0, 1)
        for i in range(4, NT):
            emit_agg_tile(0, i)
            emit_proj_tile(0, i - 2)
        emit_proj_tile(0, NT - 2)
        emit_proj_tile(0, NT - 1)
        for i in range(NT):
            emit_agg_tile(1, i)
            if i >= 2:
                emit_proj_tile(1, i - 2)
        emit_proj_tile(1, NT - 2, fast_tail=True)
        emit_proj_tile(1, NT - 1, fast_tail=True)

    nc.compile()
    return nc


def _get_nc(has_bf, has_gamma, has_beta):
    key = (has_bf, has_gamma, has_beta)
    if key not in _cache:
        _cache[key] = _build(*key)
    return _cache[key]


def _pack_band(band_mat):
    """band_mat: (N, N) ADJG^T; pack 8 diag 128-blocks then the 14 off-diag
    32-col strips (left-neighbor strip = first 32 cols, right = last 32)."""
    outp = np.zeros((128, BAND_COLS), np.float32)
    for j in range(NT):
        outp[:, 128 * j : 128 * (j + 1)] = band_mat[
            128 * j : 128 * (j + 1), 128 * j : 128 * (j + 1)
        ]
    for t, (j, i) in enumerate(OFF_BLOCKS):
        blk = band_mat[128 * j : 128 * (j + 1), 128 * i : 128 * (i + 1)]
        strip = blk[:, :SW] if i > j else blk[:, 128 - SW :]
        # verify nothing outside the strip (grid-band structure)
        outp[:, NT * 128 + SW * t : NT * 128 + SW * (t + 1)] = strip
    return outp


def _pack_rows(mat):
    """mat: (D, D') -> [128, KT*D'] with row-tile k at cols [D'*k, D'*(k+1))."""
    Dp = mat.shape[1]
    return np.ascontiguousarray(
        mat.reshape(KT, 128, Dp).transpose(1, 0, 2).reshape(128, KT * Dp)
    )


def prepare_shared(adj_weights, adj_base, node_importance, V_w, semantic_memory,
                   mix_w, mix_b, Wf, bf, gamma, beta):
    """Host-side weight folding -> shared (per-core replicated) device inputs."""
    import ml_dtypes

    bfl = ml_dtypes.bfloat16
    g = 1.0 / (1.0 + np.exp(-node_importance.astype(np.float64)))
    sig = 1.0 / (1.0 + np.exp(-adj_weights.T.astype(np.float64)))
    band_mat = (sig * adj_base.T.astype(np.float64) * g[:, None]).astype(np.float32)
    band = _pack_band(band_mat).astype(bfl)

    WfL_T = Wf[:, :D].T.astype(np.float32)           # (D, D): WfL_T[k, h] = Wf[h, k]
    sm = semantic_memory.astype(np.float32)
    C = sm @ WfL_T                                    # (D, D)
    Delta = (V_w.astype(np.float32).T - sm) @ WfL_T
    q = V_w.astype(np.float32).T @ mix_w.reshape(-1).astype(np.float32)  # (D,)

    shared = {
        "band": band,
        "cD": _pack_rows(C).astype(bfl),
        "dD": _pack_rows(Delta).astype(bfl),
        "qD": np.ascontiguousarray(q.reshape(KT, 128).T).astype(bfl),
        "mb": np.full((128, 1), float(np.asarray(mix_b).reshape(-1)[0]), np.float32),
    }
    has_bf = bool(np.any(bf != 0.0))
    has_gamma = bool(np.any(gamma != 1.0))
    has_beta = bool(np.any(beta != 0.0))
    if has_bf:
        shared["bfb"] = np.ascontiguousarray(np.tile(bf.reshape(1, D), (128, 1)))
    if has_gamma:
        shared["gab"] = np.ascontiguousarray(np.tile(gamma.reshape(1, D), (128, 1)))
    if has_beta:
        shared["beb"] = np.ascontiguousarray(np.tile(beta.reshape(1, D), (128, 1)))
    return shared, (has_bf, has_gamma, has_beta)


def kernel(
    x,
    adj_weights,
    adj_base,
    node_importance,
    V_w,
    semantic_memory,
    mix_w,
    mix_b,
    Wf,
    bf,
    gamma,
    beta,
):
    from concourse.bass_utils import run_bass_kernel_spmd

    import ml_dtypes

    bfl = ml_dtypes.bfloat16

    x = np.asarray(x, np.float32)
    shared, variant = prepare_shared(
        np.asarray(adj_weights, np.float32),
        np.asarray(adj_base, np.float32),
        np.asarray(node_importance, np.float32),
        np.asarray(V_w, np.float32),
        np.asarray(semantic_memory, np.float32),
        np.asarray(mix_w, np.float32),
        np.asarray(mix_b, np.float32),
        np.asarray(Wf, np.float32),
        np.asarray(bf, np.float32),
        np.asarray(gamma, np.float32),
        np.asarray(beta, np.float32),
    )
    nc = _get_nc(*variant)

    in_maps = []
    for c in range(NCORES):
        m = dict(shared)
        m["xN"] = np.ascontiguousarray(x[BL * c : BL * (c + 1)]).astype(bfl)
        in_maps.append(m)

    res = run_bass_kernel_spmd(nc, in_maps, core_ids=list(range(NCORES)))
    return np.concatenate(
        [res.results[c]["out"].astype(np.float32) for c in range(NCORES)], axis=0
    )


# revision 37
# speedup vs baseline: 1.0671x; 1.0351x over previous
"""Trainium2 Bass kernel for nn_AdaptiveCombinatorialComplexLayer.

Math (per batch b):
    adj   = sigmoid(adj_weights) * adj_base          # banded: diagonals {-32,-1,+1,+32}
    xg    = x * sigmoid(node_importance)[None,:,None]
    x_agg = adj @ xg
    v     = x_agg @ V_w.T ; y_pred = x_agg @ sm
    mix   = sigmoid(v @ mix_w.T + mix_b)
    x_proc= mix*v + (1-mix)*y_pred
    out   = LN(x_proc @ Wf[:, :D].T + bf) * gamma + beta

Weight-only folding (host, exact algebra):
    C     = sm @ WfL.T ; Delta = (V_w.T - sm) @ WfL.T    # WfL = Wf[:, :D]
    q     = V_w.T @ mix_w[0]
    BAND[m,n] = sigmoid(adj_weights[n,m]) * adj_base[n,m] * sigmoid(ni)[m]
              (= ADJG^T, the aggregation matrix transposed)

Device pipeline (aggregation FIRST -> one D-wide band matmul, not two):
    uT    = x^T @ BAND            # [feat, node] banded blocks
    aD    = u @ Delta ; aC = u @ C ; aq = u @ q
    mix   = sigmoid(aq + mix_b)
    z     = mix * aD + aC (+ bf)
    out   = LN(z) (* gamma + beta)          # LN stats via bn_stats/bn_aggr

Band structure exploited: for the 32x32 grid, the off-diagonal 128x128 tile
blocks of BAND have nonzeros only in a 32-wide column strip (boundary rows),
so they are packed and matmul'ed as [128, 32] strips.

Sharding: pure data-parallel over batch, 2 batches per core, weights replicated.
"""

import numpy as np

B, N, D, G = 16, 1024, 512, 32
NCORES = 8
BL = B // NCORES          # batches per core
NT = N // 128             # 8 node tiles of 128
KT = D // 128             # 4 feature tiles of 128
LN_EPS = 1e-5
SW = 32                   # off-diagonal strip width

# band packed as per-tile groups [diag(i), strip(i-1,i), strip(i+1,i)] so a
# short DMA prefix unlocks the first aggregation tiles
def _grp_off(i):
    nstrips = 0 if i == 0 else 2 * i - 1
    return 128 * i + SW * nstrips


BAND_COLS = _grp_off(NT - 1) + 128 + SW   # last group: diag + left strip

_cache = {}


def _build(has_bf, has_gamma, has_beta):
    from contextlib import ExitStack

    import concourse.bass as bass
    import concourse.tile as tile
    from concourse import bacc, mybir

    f32 = mybir.dt.float32
    bf16 = mybir.dt.bfloat16
    i32 = mybir.dt.int32
    AF = mybir.ActivationFunctionType
    OP = mybir.AluOpType

    nc = bacc.Bacc(
        "TRN2",
        target_bir_lowering=False,
        debug=False,
        num_devices=NCORES,
    )

    xN = nc.dram_tensor("xN", [BL, N, D], bf16, kind="ExternalInput")
    band = nc.dram_tensor("band", [128, BAND_COLS], bf16, kind="ExternalInput")
    cD = nc.dram_tensor("cD", [128, KT * D], bf16, kind="ExternalInput")
    dD = nc.dram_tensor("dD", [128, KT * D], bf16, kind="ExternalInput")
    qD = nc.dram_tensor("qD", [128, KT], bf16, kind="ExternalInput")
    mb = nc.dram_tensor("mb", [128, 1], f32, kind="ExternalInput")
    if has_bf:
        bfb = nc.dram_tensor("bfb", [128, D], f32, kind="ExternalInput")
    if has_gamma:
        gab = nc.dram_tensor("gab", [128, D], f32, kind="ExternalInput")
    if has_beta:
        beb = nc.dram_tensor("beb", [128, D], f32, kind="ExternalInput")
    out = nc.dram_tensor("out", [BL, N, D], bf16, kind="ExternalOutput")

    def diag_ap(i):
        o = _grp_off(i)
        return band_sb[:, o : o + 128]

    def off_ap(j, i):
        o = _grp_off(i) + 128
        if j > i and i > 0:
            o += SW          # right strip sits after the left strip
        return band_sb[:, o : o + SW]

    with ExitStack() as ctx:
        tc = ctx.enter_context(tile.TileContext(nc))
        const = ctx.enter_context(tc.tile_pool(name="const", bufs=1))

        # ---- persistent SBUF tensors ----
        mb_sb = const.tile([128, 1], f32)
        magic = const.tile([128, 2], i32)     # 0x5f3759df for NR rsqrt
        nc.vector.memset(magic[:], 0x5F3759DF)
        junk = const.tile([128, D], bf16)     # PE p-state warmup operand
        nc.vector.memset(junk[:], 0.0)
        eps_sb = const.tile([128, 1], f32)
        nc.vector.memset(eps_sb[:], LN_EPS)
        band_sb = const.tile([128, BAND_COLS], bf16)
        c_bf = const.tile([128, KT * D], bf16)
        d_bf = const.tile([128, KT * D], bf16)
        q_bf = const.tile([128, KT], bf16)
        if has_bf:
            bf_sb = const.tile([128, D], f32)
            nc.sync.dma_start(bf_sb[:], bfb[:])
        if has_gamma:
            ga_sb = const.tile([128, D], f32)
            nc.sync.dma_start(ga_sb[:], gab[:])
        if has_beta:
            be_sb = const.tile([128, D], f32)
            nc.sync.dma_start(be_sb[:], beb[:])

        xpool = ctx.enter_context(tc.tile_pool(name="xpool", bufs=BL))
        upool = ctx.enter_context(tc.tile_pool(name="upool", bufs=BL))
        xsb = []
        for b in range(BL):
            xsb.append(xpool.tile([128, NT * D], bf16, tag=f"x{b}", name=f"x{b}"))
        usb = [upool.tile([128, NT * D], bf16, tag=f"u{b}", name=f"u{b}")
               for b in range(BL)]

        def load_x(b, jlo, jhi):
            nc.sync.dma_start(
                xsb[b][:, D * jlo : D * jhi].rearrange("p (j d) -> p j d", d=D),
                xN[b, 128 * jlo : 128 * jhi].rearrange("(j p) d -> p j d", p=128),
            )

        # ---- DMA issue order == serial transfer order on the DMA pipe:
        # gate b0 aggregation first, then the weights for the projections,
        # then the rest of x.
        nc.sync.dma_start(band_sb[:], band[:])
        load_x(0, 0, 4)
        nc.sync.dma_start(
            c_bf[:].rearrange("p (k c) -> p k c", k=KT),
            cD[:].rearrange("p (k c) -> p k c", k=KT),
        )
        nc.sync.dma_start(q_bf[:], qD[:])
        nc.sync.dma_start(mb_sb[:], mb[:])
        nc.sync.dma_start(
            d_bf[:].rearrange("p (k c) -> p k c", k=KT),
            dD[:].rearrange("p (k c) -> p k c", k=KT),
        )
        load_x(0, 4, 8)
        load_x(1, 0, 4)
        load_x(1, 4, 8)

        # ---- PSUM pools: 8 banks ----
        psU = ctx.enter_context(tc.tile_pool(name="psU", bufs=2, space="PSUM"))
        psA = ctx.enter_context(tc.tile_pool(name="psA", bufs=2, space="PSUM"))
        psB = ctx.enter_context(tc.tile_pool(name="psB", bufs=2, space="PSUM"))
        psS = ctx.enter_context(tc.tile_pool(name="psS", bufs=2, space="PSUM"))

        epi = ctx.enter_context(tc.tile_pool(name="epi", bufs=4))
        zpool = ctx.enter_context(tc.tile_pool(name="zpool", bufs=4))
        opool = ctx.enter_context(tc.tile_pool(name="opool", bufs=3))

        def emit_warm(n):
            # keep the PE p-state ramp hot across known DMA-pacing stalls;
            # writes are never read (recycled tags)
            for _ in range(n):
                pj = psA.tile([128, D], f32, tag="bigA", name="pj")
                nc.tensor.matmul(pj[:], junk[:, :128], junk[:], start=True, stop=True)

        def emit_agg_tile(b, i, evict_act=True):
            """uT tile i of batch b -> usb[b][:, 512i + 128k] (bf16).

            Off-diagonal neighbor blocks touch only a 32-col strip of the
            output: left neighbor -> cols [0,32), right -> cols [96,128)."""
            pu = psU.tile([128, D], f32, tag="u")
            xl = xsb[b]
            for k in range(KT):
                ks = slice(128 * k, 128 * (k + 1))
                lhs_i = xl[:, D * i + 128 * k : D * i + 128 * (k + 1)]
                base = 128 * k
                # segments of the 128 output cols: (lo, hi, with_off, j_off)
                segs = []
                if i > 0:
                    segs.append((0, SW, True, i - 1))
                    segs.append((SW, 128 if i == NT - 1 else 128 - SW, False, 0))
                else:
                    segs.append((0, 128 - SW, False, 0))
                if i < NT - 1:
                    segs.append((128 - SW, 128, True, i + 1))
                for lo, hi, with_off, joff in segs:
                    osl = pu[:, base + lo : base + hi]
                    nc.tensor.matmul(
                        osl, lhs_i, diag_ap(i)[:, lo:hi],
                        start=True, stop=not with_off,
                    )
                    if with_off:
                        lhs_o = xl[:, D * joff + 128 * k : D * joff + 128 * (k + 1)]
                        nc.tensor.matmul(
                            osl, lhs_o, off_ap(joff, i), start=False, stop=True
                        )
            if evict_act:
                nc.scalar.activation(
                    usb[b][:, D * i : D * (i + 1)], pu[:], AF.Copy
                )
            else:
                nc.vector.tensor_copy(usb[b][:, D * i : D * (i + 1)], pu[:])

        pending_ot = []
        pair_state = {}

        def flush_ot():
            # deferred one tile so the ACT queue never head-blocks on the
            # (late-ready) scale-shift while the next tile's mix is ready
            while pending_ot:
                b, i, z, y, nmr = pending_ot.pop(0)
                ot = opool.tile([128, D], bf16, tag="ot", name="ot")
                nc.scalar.activation(
                    ot[:, : D // 2], z[:, : D // 2], AF.Identity,
                    bias=nmr[:], scale=y[:],
                )
                nc.vector.tensor_scalar(
                    ot[:, D // 2 :], z[:, D // 2 :], y[:], nmr[:],
                    OP.mult, OP.add,
                )
                if has_gamma:
                    nc.vector.tensor_tensor(ot[:], ot[:], ga_sb[:], OP.mult)
                if has_beta:
                    nc.vector.tensor_tensor(ot[:], ot[:], be_sb[:], OP.add)
                nc.sync.dma_start(out[b, 128 * i : 128 * (i + 1), :], ot[:])

        def emit_proj_tile(b, i, fast_tail=False):
            """Project uT tile i through q/C/Delta (q first: its stop releases
            the mix sigmoid early), then the fused epilogue: mix-combine,
            bn LN stats, NR rsqrt, deferred scale-shift + store.

            fast_tail: split the C/Delta projections and the epilogue into
            column halves so the first half's DVE chain overlaps the second
            half's matmuls (used for the last tiles, where no later PE work
            hides the epilogue)."""
            pa_c = psB.tile([128, D], f32, tag="bigB")
            pa_q = psS.tile([128, 1], f32, tag="sm")
            for k in range(KT):
                lhsT = usb[b][:, D * i + 128 * k : D * i + 128 * (k + 1)]
                nc.tensor.matmul(
                    pa_q[:], lhsT, q_bf[:, k : k + 1],
                    start=k == 0, stop=k == KT - 1,
                )
            halves = (
                [(0, D // 2), (D // 2, D)] if fast_tail else [(0, D)]
            )
            # separate PSUM tile per Delta half: a matmul start into a tile
            # that a prior combine read from serializes (tile-granular WAR)
            pa_ds = [psA.tile([128, D], f32, tag="bigA", name="pa_d")
                     for _ in halves]
            for k in range(KT):
                lhsT = usb[b][:, D * i + 128 * k : D * i + 128 * (k + 1)]
                nc.tensor.matmul(
                    pa_c[:], lhsT, c_bf[:, D * k : D * (k + 1)],
                    start=k == 0, stop=k == KT - 1,
                )
            mix = epi.tile([128, 1], f32, tag="mix")
            nc.scalar.activation(
                mix[:], pa_q[:], AF.Sigmoid, bias=mb_sb[:], scale=1.0
            )
            # HW: only one non-scalar PSUM operand per instruction
            csb = epi.tile([128, D], f32, tag="csb")
            nc.scalar.activation(csb[:], pa_c[:], AF.Copy)
            flush_ot()
            z = zpool.tile([128, D], f32, tag="z")
            HD = D // 2
            sums = epi.tile([128, len(halves)], f32, tag="sums")
            for h, (lo, hi) in enumerate(halves):
                pa_d = pa_ds[h]
                for k in range(KT):
                    lhsT = usb[b][:, D * i + 128 * k : D * i + 128 * (k + 1)]
                    nc.tensor.matmul(
                        pa_d[:, : hi - lo], lhsT, d_bf[:, D * k + lo : D * k + hi],
                        start=k == 0, stop=k == KT - 1,
                    )
                nc.vector.scalar_tensor_tensor(
                    z[:, lo:hi], pa_d[:, : hi - lo], mix[:], csb[:, lo:hi],
                    OP.mult, OP.add, accum_out=sums[:, h : h + 1],
                )
                if has_bf:
                    nc.vector.tensor_tensor(
                        z[:, lo:hi], z[:, lo:hi], bf_sb[:, lo:hi], OP.add
                    )
                    nc.vector.tensor_scalar(
                        z[:, lo:hi], z[:, lo:hi], 0.0, None, OP.add,
                        accum_out=sums[:, h : h + 1],
                    )
            # sum of squares: first half on ACT (Square), second on DVE
            sq = epi.tile([128, 2], f32, tag="sq")
            zsq = epi.tile([128, HD], f32, tag="zsq")
            nc.scalar.activation(
                zsq[:], z[:, :HD], AF.Square, accum_out=sq[:, 0:1]
            )
            zsq2 = epi.tile([128, HD], f32, tag="zsq2")
            nc.vector.scalar_tensor_tensor(
                zsq2[:], z[:, HD:], 1.0, z[:, HD:], OP.mult, OP.mult,
                accum_out=sq[:, 1:2],
            )
            eng = nc.vector
            mean = epi.tile([128, 1], f32, tag="mean")
            if len(halves) == 1:
                eng.tensor_scalar(mean[:], sums[:, 0:1], 1.0 / D, None, OP.mult)
            else:
                eng.tensor_tensor(
                    sums[:, 0:1], sums[:, 0:1], sums[:, 1:2], OP.add
                )
                eng.tensor_scalar(mean[:], sums[:, 0:1], 1.0 / D, None, OP.mult)
            eng.tensor_tensor(sq[:, 0:1], sq[:, 0:1], sq[:, 1:2], OP.add)
            m2 = epi.tile([128, 1], f32, tag="m2")
            # m2 - eps, so va = sq/D - (m2 - eps) lands in one op
            eng.scalar_tensor_tensor(
                m2[:], mean[:], mean[:], eps_sb[:], OP.mult, OP.subtract
            )
            va = epi.tile([128, 1], f32, tag="va")
            eng.scalar_tensor_tensor(
                va[:], sq[:, 0:1], 1.0 / D, m2[:], OP.mult, OP.subtract
            )
            ih = epi.tile([128, 1], i32, tag="ih")
            eng.tensor_scalar(ih[:], va[:].bitcast(i32), 1, None, OP.arith_shift_right)
            y = epi.tile([128, 1], f32, tag="y")
            eng.scalar_tensor_tensor(
                y[:].bitcast(i32), magic[:, :1], 0, ih[:], OP.bypass, OP.subtract
            )
            t1 = epi.tile([128, 1], f32, tag="t1")
            eng.tensor_tensor(t1[:], y[:], y[:], OP.mult)
            eng.tensor_tensor(t1[:], t1[:], va[:], OP.mult)
            eng.tensor_scalar(t1[:], t1[:], -0.5, 1.5, OP.mult, OP.add)
            eng.tensor_tensor(y[:], y[:], t1[:], OP.mult)
            nmr = epi.tile([128, 1], f32, tag="nmr")
            eng.tensor_scalar(nmr[:], mean[:], y[:], -1.0, OP.mult, OP.mult)
            if fast_tail:
                # scale-shift halves in parallel on ACT + DVE, store at once
                ot = opool.tile([128, D], bf16, tag="ot", name="ot")
                nc.scalar.activation(
                    ot[:, :HD], z[:, :HD], AF.Identity,
                    bias=nmr[:], scale=y[:],
                )
                nc.vector.tensor_scalar(
                    ot[:, HD:], z[:, HD:], y[:], nmr[:], OP.mult, OP.add
                )
                if has_gamma:
                    nc.vector.tensor_tensor(ot[:], ot[:], ga_sb[:], OP.mult)
                if has_beta:
                    nc.vector.tensor_tensor(ot[:], ot[:], be_sb[:], OP.add)
                nc.sync.dma_start(out[b, 128 * i : 128 * (i + 1), :], ot[:])
            else:
                pending_ot.append((b, i, z, y, nmr))

        # ---- schedule ----
        # warm PE p-state over the initial DMA wait; then one agg or proj
        # tile per step so the ACT/DVE epilogue+evict stream stays paced with
        # PE (bursts at phase boundaries starve PE on evict recycling).
        emit_warm(8)
        for i in range(4):
            emit_agg_tile(0, i)
        emit_proj_tile(0, 0)
        emit_proj_tile(0, 1)
        for i in range(4, NT):
            emit_agg_tile(0, i, evict_act=i % 2 == 0)
            emit_proj_tile(0, i - 2)
        for i in range(2):
            emit_agg_tile(1, i, evict_act=i % 2 == 0)
            emit_proj_tile(0, NT - 2 + i)
        for i in range(2, NT):
            emit_agg_tile(1, i, evict_act=i % 2 == 0)
            emit_proj_tile(1, i - 2)
        emit_proj_tile(1, NT - 2)
        emit_proj_tile(1, NT - 1, fast_tail=True)

    nc.compile()
    return nc


def _get_nc(has_bf, has_gamma, has_beta):
    key = (has_bf, has_gamma, has_beta)
    if key not in _cache:
        _cache[key] = _build(*key)
    return _cache[key]


def _pack_band(band_mat):
    """band_mat: (N, N) ADJG^T; per-tile groups [diag(i), strip(i-1,i),
    strip(i+1,i)] (strip = the 32 nonzero cols of the off-diag block)."""
    outp = np.zeros((128, BAND_COLS), np.float32)
    for i in range(NT):
        o = _grp_off(i)
        outp[:, o : o + 128] = band_mat[
            128 * i : 128 * (i + 1), 128 * i : 128 * (i + 1)
        ]
        o += 128
        if i > 0:   # left neighbor j=i-1: nonzeros in first 32 cols
            blk = band_mat[128 * (i - 1) : 128 * i, 128 * i : 128 * (i + 1)]
            outp[:, o : o + SW] = blk[:, :SW]
            o += SW
        if i < NT - 1:   # right neighbor j=i+1: nonzeros in last 32 cols
            blk = band_mat[128 * (i + 1) : 128 * (i + 2), 128 * i : 128 * (i + 1)]
            outp[:, o : o + SW] = blk[:, 128 - SW :]
    return outp


def _pack_rows(mat):
    """mat: (D, D') -> [128, KT*D'] with row-tile k at cols [D'*k, D'*(k+1))."""
    Dp = mat.shape[1]
    return np.ascontiguousarray(
        mat.reshape(KT, 128, Dp).transpose(1, 0, 2).reshape(128, KT * Dp)
    )


def prepare_shared(adj_weights, adj_base, node_importance, V_w, semantic_memory,
                   mix_w, mix_b, Wf, bf, gamma, beta):
    """Host-side weight folding -> shared (per-core replicated) device inputs."""
    import ml_dtypes

    bfl = ml_dtypes.bfloat16
    g = 1.0 / (1.0 + np.exp(-node_importance.astype(np.float64)))
    sig = 1.0 / (1.0 + np.exp(-adj_weights.T.astype(np.float64)))
    band_mat = (sig * adj_base.T.astype(np.float64) * g[:, None]).astype(np.float32)
    band = _pack_band(band_mat).astype(bfl)

    WfL_T = Wf[:, :D].T.astype(np.float32)           # (D, D): WfL_T[k, h] = Wf[h, k]
    sm = semantic_memory.astype(np.float32)
    C = sm @ WfL_T                                    # (D, D)
    Delta = (V_w.astype(np.float32).T - sm) @ WfL_T
    q = V_w.astype(np.float32).T @ mix_w.reshape(-1).astype(np.float32)  # (D,)

    shared = {
        "band": band,
        "cD": _pack_rows(C).astype(bfl),
        "dD": _pack_rows(Delta).astype(bfl),
        "qD": np.ascontiguousarray(q.reshape(KT, 128).T).astype(bfl),
        "mb": np.full((128, 1), float(np.asarray(mix_b).reshape(-1)[0]), np.float32),
    }
    has_bf = bool(np.any(bf != 0.0))
    has_gamma = bool(np.any(gamma != 1.0))
    has_beta = bool(np.any(beta != 0.0))
    if has_bf:
        shared["bfb"] = np.ascontiguousarray(np.tile(bf.reshape(1, D), (128, 1)))
    if has_gamma:
        shared["gab"] = np.ascontiguousarray(np.tile(gamma.reshape(1, D), (128, 1)))
    if has_beta:
        shared["beb"] = np.ascontiguousarray(np.tile(beta.reshape(1, D), (128, 1)))
    return shared, (has_bf, has_gamma, has_beta)


def kernel(
    x,
    adj_weights,
    adj_base,
    node_importance,
    V_w,
    semantic_memory,
    mix_w,
    mix_b,
    Wf,
    bf,
    gamma,
    beta,
):
    from concourse.bass_utils import run_bass_kernel_spmd

    import ml_dtypes

    bfl = ml_dtypes.bfloat16

    x = np.asarray(x, np.float32)
    shared, variant = prepare_shared(
        np.asarray(adj_weights, np.float32),
        np.asarray(adj_base, np.float32),
        np.asarray(node_importance, np.float32),
        np.asarray(V_w, np.float32),
        np.asarray(semantic_memory, np.float32),
        np.asarray(mix_w, np.float32),
        np.asarray(mix_b, np.float32),
        np.asarray(Wf, np.float32),
        np.asarray(bf, np.float32),
        np.asarray(gamma, np.float32),
        np.asarray(beta, np.float32),
    )
    nc = _get_nc(*variant)

    in_maps = []
    for c in range(NCORES):
        m = dict(shared)
        m["xN"] = np.ascontiguousarray(x[BL * c : BL * (c + 1)]).astype(bfl)
        in_maps.append(m)

    res = run_bass_kernel_spmd(nc, in_maps, core_ids=list(range(NCORES)))
    return np.concatenate(
        [res.results[c]["out"].astype(np.float32) for c in range(NCORES)], axis=0
    )


# revision 38
# speedup vs baseline: 1.0679x; 1.0008x over previous
"""Trainium2 Bass kernel for nn_AdaptiveCombinatorialComplexLayer.

Math (per batch b):
    adj   = sigmoid(adj_weights) * adj_base          # banded: diagonals {-32,-1,+1,+32}
    xg    = x * sigmoid(node_importance)[None,:,None]
    x_agg = adj @ xg
    v     = x_agg @ V_w.T ; y_pred = x_agg @ sm
    mix   = sigmoid(v @ mix_w.T + mix_b)
    x_proc= mix*v + (1-mix)*y_pred
    out   = LN(x_proc @ Wf[:, :D].T + bf) * gamma + beta

Weight-only folding (host, exact algebra):
    C     = sm @ WfL.T ; Delta = (V_w.T - sm) @ WfL.T    # WfL = Wf[:, :D]
    q     = V_w.T @ mix_w[0]
    BAND[m,n] = sigmoid(adj_weights[n,m]) * adj_base[n,m] * sigmoid(ni)[m]
              (= ADJG^T, the aggregation matrix transposed)

Device pipeline (aggregation FIRST -> one D-wide band matmul, not two):
    uT    = x^T @ BAND            # [feat, node] banded blocks
    aD    = u @ Delta ; aC = u @ C ; aq = u @ q
    mix   = sigmoid(aq + mix_b)
    z     = mix * aD + aC (+ bf)
    out   = LN(z) (* gamma + beta)          # LN stats via bn_stats/bn_aggr

Band structure exploited: for the 32x32 grid, the off-diagonal 128x128 tile
blocks of BAND have nonzeros only in a 32-wide column strip (boundary rows),
so they are packed and matmul'ed as [128, 32] strips.

Sharding: pure data-parallel over batch, 2 batches per core, weights replicated.
"""

import numpy as np

B, N, D, G = 16, 1024, 512, 32
NCORES = 8
BL = B // NCORES          # batches per core
NT = N // 128             # 8 node tiles of 128
KT = D // 128             # 4 feature tiles of 128
LN_EPS = 1e-5
SW = 32                   # off-diagonal strip width

# band packed as per-tile groups [diag(i), strip(i-1,i), strip(i+1,i)] so a
# short DMA prefix unlocks the first aggregation tiles
def _grp_off(i):
    nstrips = 0 if i == 0 else 2 * i - 1
    return 128 * i + SW * nstrips


BAND_COLS = _grp_off(NT - 1) + 128 + SW   # last group: diag + left strip

_cache = {}


def _build(has_bf, has_gamma, has_beta):
    from contextlib import ExitStack

    import concourse.bass as bass
    import concourse.tile as tile
    from concourse import bacc, mybir

    f32 = mybir.dt.float32
    bf16 = mybir.dt.bfloat16
    i32 = mybir.dt.int32
    AF = mybir.ActivationFunctionType
    OP = mybir.AluOpType

    nc = bacc.Bacc(
        "TRN2",
        target_bir_lowering=False,
        debug=False,
        num_devices=NCORES,
    )

    xN = nc.dram_tensor("xN", [BL, N, D], bf16, kind="ExternalInput")
    band = nc.dram_tensor("band", [128, BAND_COLS], bf16, kind="ExternalInput")
    cD = nc.dram_tensor("cD", [128, KT * D], bf16, kind="ExternalInput")
    dD = nc.dram_tensor("dD", [128, KT * D], bf16, kind="ExternalInput")
    qD = nc.dram_tensor("qD", [128, KT], bf16, kind="ExternalInput")
    mb = nc.dram_tensor("mb", [128, 1], f32, kind="ExternalInput")
    if has_bf:
        bfb = nc.dram_tensor("bfb", [128, D], f32, kind="ExternalInput")
    if has_gamma:
        gab = nc.dram_tensor("gab", [128, D], f32, kind="ExternalInput")
    if has_beta:
        beb = nc.dram_tensor("beb", [128, D], f32, kind="ExternalInput")
    out = nc.dram_tensor("out", [BL, N, D], bf16, kind="ExternalOutput")

    def diag_ap(i):
        o = _grp_off(i)
        return band_sb[:, o : o + 128]

    def off_ap(j, i):
        o = _grp_off(i) + 128
        if j > i and i > 0:
            o += SW          # right strip sits after the left strip
        return band_sb[:, o : o + SW]

    with ExitStack() as ctx:
        tc = ctx.enter_context(tile.TileContext(nc))
        const = ctx.enter_context(tc.tile_pool(name="const", bufs=1))

        # ---- persistent SBUF tensors ----
        mb_sb = const.tile([128, 1], f32)
        magic = const.tile([128, 2], i32)     # 0x5f3759df for NR rsqrt
        nc.vector.memset(magic[:], 0x5F3759DF)
        junk = const.tile([128, D], bf16)     # PE p-state warmup operand
        nc.vector.memset(junk[:], 0.0)
        eps_sb = const.tile([128, 1], f32)
        nc.vector.memset(eps_sb[:], LN_EPS)
        band_sb = const.tile([128, BAND_COLS], bf16)
        c_bf = const.tile([128, KT * D], bf16)
        d_bf = const.tile([128, KT * D], bf16)
        q_bf = const.tile([128, KT], bf16)
        if has_bf:
            bf_sb = const.tile([128, D], f32)
            nc.sync.dma_start(bf_sb[:], bfb[:])
        if has_gamma:
            ga_sb = const.tile([128, D], f32)
            nc.sync.dma_start(ga_sb[:], gab[:])
        if has_beta:
            be_sb = const.tile([128, D], f32)
            nc.sync.dma_start(be_sb[:], beb[:])

        xpool = ctx.enter_context(tc.tile_pool(name="xpool", bufs=BL))
        upool = ctx.enter_context(tc.tile_pool(name="upool", bufs=BL))
        xsb = []
        for b in range(BL):
            xsb.append(xpool.tile([128, NT * D], bf16, tag=f"x{b}", name=f"x{b}"))
        usb = [upool.tile([128, NT * D], bf16, tag=f"u{b}", name=f"u{b}")
               for b in range(BL)]

        def load_x(b, jlo, jhi):
            nc.sync.dma_start(
                xsb[b][:, D * jlo : D * jhi].rearrange("p (j d) -> p j d", d=D),
                xN[b, 128 * jlo : 128 * jhi].rearrange("(j p) d -> p j d", p=128),
            )

        # ---- DMA issue order == serial transfer order on the DMA pipe:
        # gate b0 aggregation first, then the weights for the projections,
        # then the rest of x.
        nc.sync.dma_start(band_sb[:], band[:])
        load_x(0, 0, 4)
        nc.sync.dma_start(
            c_bf[:].rearrange("p (k c) -> p k c", k=KT),
            cD[:].rearrange("p (k c) -> p k c", k=KT),
        )
        nc.sync.dma_start(q_bf[:], qD[:])
        nc.sync.dma_start(mb_sb[:], mb[:])
        nc.sync.dma_start(
            d_bf[:].rearrange("p (k c) -> p k c", k=KT),
            dD[:].rearrange("p (k c) -> p k c", k=KT),
        )
        load_x(0, 4, 8)
        load_x(1, 0, 4)
        load_x(1, 4, 8)

        # ---- PSUM pools: 8 banks ----
        psU = ctx.enter_context(tc.tile_pool(name="psU", bufs=2, space="PSUM"))
        psA = ctx.enter_context(tc.tile_pool(name="psA", bufs=2, space="PSUM"))
        psB = ctx.enter_context(tc.tile_pool(name="psB", bufs=2, space="PSUM"))
        psS = ctx.enter_context(tc.tile_pool(name="psS", bufs=2, space="PSUM"))

        epi = ctx.enter_context(tc.tile_pool(name="epi", bufs=4))
        zpool = ctx.enter_context(tc.tile_pool(name="zpool", bufs=4))
        opool = ctx.enter_context(tc.tile_pool(name="opool", bufs=3))

        def emit_warm(n):
            # keep the PE p-state ramp hot across known DMA-pacing stalls;
            # writes are never read (recycled tags)
            for _ in range(n):
                pj = psA.tile([128, D], f32, tag="bigA", name="pj")
                nc.tensor.matmul(pj[:], junk[:, :128], junk[:], start=True, stop=True)

        def emit_agg_tile(b, i, evict_act=True):
            """uT tile i of batch b -> usb[b][:, 512i + 128k] (bf16).

            Off-diagonal neighbor blocks touch only a 32-col strip of the
            output: left neighbor -> cols [0,32), right -> cols [96,128)."""
            pu = psU.tile([128, D], f32, tag="u")
            xl = xsb[b]
            for k in range(KT):
                ks = slice(128 * k, 128 * (k + 1))
                lhs_i = xl[:, D * i + 128 * k : D * i + 128 * (k + 1)]
                base = 128 * k
                # segments of the 128 output cols: (lo, hi, with_off, j_off)
                segs = []
                if i > 0:
                    segs.append((0, SW, True, i - 1))
                    segs.append((SW, 128 if i == NT - 1 else 128 - SW, False, 0))
                else:
                    segs.append((0, 128 - SW, False, 0))
                if i < NT - 1:
                    segs.append((128 - SW, 128, True, i + 1))
                for lo, hi, with_off, joff in segs:
                    osl = pu[:, base + lo : base + hi]
                    nc.tensor.matmul(
                        osl, lhs_i, diag_ap(i)[:, lo:hi],
                        start=True, stop=not with_off,
                    )
                    if with_off:
                        lhs_o = xl[:, D * joff + 128 * k : D * joff + 128 * (k + 1)]
                        nc.tensor.matmul(
                            osl, lhs_o, off_ap(joff, i), start=False, stop=True
                        )
            if evict_act:
                nc.scalar.activation(
                    usb[b][:, D * i : D * (i + 1)], pu[:], AF.Copy
                )
            else:
                nc.vector.tensor_copy(usb[b][:, D * i : D * (i + 1)], pu[:])

        pending_ot = []
        pair_state = {}

        def flush_ot():
            # deferred one tile so the ACT queue never head-blocks on the
            # (late-ready) scale-shift while the next tile's mix is ready
            while pending_ot:
                b, i, z, y, nmr = pending_ot.pop(0)
                ot = opool.tile([128, D], bf16, tag="ot", name="ot")
                nc.scalar.activation(
                    ot[:, : D // 2], z[:, : D // 2], AF.Identity,
                    bias=nmr[:], scale=y[:],
                )
                nc.vector.tensor_scalar(
                    ot[:, D // 2 :], z[:, D // 2 :], y[:], nmr[:],
                    OP.mult, OP.add,
                )
                if has_gamma:
                    nc.vector.tensor_tensor(ot[:], ot[:], ga_sb[:], OP.mult)
                if has_beta:
                    nc.vector.tensor_tensor(ot[:], ot[:], be_sb[:], OP.add)
                nc.sync.dma_start(out[b, 128 * i : 128 * (i + 1), :], ot[:])

        def emit_proj_tile(b, i, fast_tail=False):
            """Project uT tile i through q/C/Delta (q first: its stop releases
            the mix sigmoid early), then the fused epilogue: mix-combine,
            bn LN stats, NR rsqrt, deferred scale-shift + store.

            fast_tail: split the C/Delta projections and the epilogue into
            column halves so the first half's DVE chain overlaps the second
            half's matmuls (used for the last tiles, where no later PE work
            hides the epilogue)."""
            pa_c = psB.tile([128, D], f32, tag="bigB")
            pa_q = psS.tile([128, 1], f32, tag="sm")
            for k in range(KT):
                lhsT = usb[b][:, D * i + 128 * k : D * i + 128 * (k + 1)]
                nc.tensor.matmul(
                    pa_q[:], lhsT, q_bf[:, k : k + 1],
                    start=k == 0, stop=k == KT - 1,
                )
            halves = (
                [(0, D // 2), (D // 2, D)] if fast_tail else [(0, D)]
            )
            # separate PSUM tile per Delta half: a matmul start into a tile
            # that a prior combine read from serializes (tile-granular WAR)
            pa_ds = [psA.tile([128, D], f32, tag="bigA", name="pa_d")
                     for _ in halves]
            for k in range(KT):
                lhsT = usb[b][:, D * i + 128 * k : D * i + 128 * (k + 1)]
                nc.tensor.matmul(
                    pa_c[:], lhsT, c_bf[:, D * k : D * (k + 1)],
                    start=k == 0, stop=k == KT - 1,
                )
            mix = epi.tile([128, 1], f32, tag="mix")
            nc.scalar.activation(
                mix[:], pa_q[:], AF.Sigmoid, bias=mb_sb[:], scale=1.0
            )
            # HW: only one non-scalar PSUM operand per instruction
            csb = epi.tile([128, D], f32, tag="csb")
            nc.scalar.activation(csb[:], pa_c[:], AF.Copy)
            flush_ot()
            z = zpool.tile([128, D], f32, tag="z")
            HD = D // 2
            sums = epi.tile([128, len(halves)], f32, tag="sums")
            for h, (lo, hi) in enumerate(halves):
                pa_d = pa_ds[h]
                for k in range(KT):
                    lhsT = usb[b][:, D * i + 128 * k : D * i + 128 * (k + 1)]
                    nc.tensor.matmul(
                        pa_d[:, : hi - lo], lhsT, d_bf[:, D * k + lo : D * k + hi],
                        start=k == 0, stop=k == KT - 1,
                    )
                nc.vector.scalar_tensor_tensor(
                    z[:, lo:hi], pa_d[:, : hi - lo], mix[:], csb[:, lo:hi],
                    OP.mult, OP.add, accum_out=sums[:, h : h + 1],
                )
                if has_bf:
                    nc.vector.tensor_tensor(
                        z[:, lo:hi], z[:, lo:hi], bf_sb[:, lo:hi], OP.add
                    )
                    nc.vector.tensor_scalar(
                        z[:, lo:hi], z[:, lo:hi], 0.0, None, OP.add,
                        accum_out=sums[:, h : h + 1],
                    )
            # sum of squares: first half on ACT (Square), second on DVE
            sq = epi.tile([128, 2], f32, tag="sq")
            zsq = epi.tile([128, HD], f32, tag="zsq")
            nc.scalar.activation(
                zsq[:], z[:, :HD], AF.Square, accum_out=sq[:, 0:1]
            )
            zsq2 = epi.tile([128, HD], f32, tag="zsq2")
            nc.vector.scalar_tensor_tensor(
                zsq2[:], z[:, HD:], 1.0, z[:, HD:], OP.mult, OP.mult,
                accum_out=sq[:, 1:2],
            )
            eng = nc.vector
            mean = epi.tile([128, 1], f32, tag="mean")
            if len(halves) == 1:
                eng.tensor_scalar(mean[:], sums[:, 0:1], 1.0 / D, None, OP.mult)
            else:
                eng.tensor_tensor(
                    sums[:, 0:1], sums[:, 0:1], sums[:, 1:2], OP.add
                )
                eng.tensor_scalar(mean[:], sums[:, 0:1], 1.0 / D, None, OP.mult)
            eng.tensor_tensor(sq[:, 0:1], sq[:, 0:1], sq[:, 1:2], OP.add)
            m2 = epi.tile([128, 1], f32, tag="m2")
            # m2 - eps, so va = sq/D - (m2 - eps) lands in one op
            eng.scalar_tensor_tensor(
                m2[:], mean[:], mean[:], eps_sb[:], OP.mult, OP.subtract
            )
            va = epi.tile([128, 1], f32, tag="va")
            eng.scalar_tensor_tensor(
                va[:], sq[:, 0:1], 1.0 / D, m2[:], OP.mult, OP.subtract
            )
            ih = epi.tile([128, 1], i32, tag="ih")
            eng.tensor_scalar(ih[:], va[:].bitcast(i32), 1, None, OP.arith_shift_right)
            y = epi.tile([128, 1], f32, tag="y")
            eng.scalar_tensor_tensor(
                y[:].bitcast(i32), magic[:, :1], 0, ih[:], OP.bypass, OP.subtract
            )
            t1 = epi.tile([128, 1], f32, tag="t1")
            eng.tensor_tensor(t1[:], y[:], y[:], OP.mult)
            eng.tensor_tensor(t1[:], t1[:], va[:], OP.mult)
            eng.tensor_scalar(t1[:], t1[:], -0.5, 1.5, OP.mult, OP.add)
            eng.tensor_tensor(y[:], y[:], t1[:], OP.mult)
            nmr = epi.tile([128, 1], f32, tag="nmr")
            eng.tensor_scalar(nmr[:], mean[:], y[:], -1.0, OP.mult, OP.mult)
            if fast_tail:
                # scale-shift halves in parallel on ACT + DVE, store at once
                ot = opool.tile([128, D], bf16, tag="ot", name="ot")
                nc.scalar.activation(
                    ot[:, :HD], z[:, :HD], AF.Identity,
                    bias=nmr[:], scale=y[:],
                )
                nc.vector.tensor_scalar(
                    ot[:, HD:], z[:, HD:], y[:], nmr[:], OP.mult, OP.add
                )
                if has_gamma:
                    nc.vector.tensor_tensor(ot[:], ot[:], ga_sb[:], OP.mult)
                if has_beta:
                    nc.vector.tensor_tensor(ot[:], ot[:], be_sb[:], OP.add)
                nc.sync.dma_start(out[b, 128 * i : 128 * (i + 1), :], ot[:])
            else:
                pending_ot.append((b, i, z, y, nmr))

        # ---- schedule ----
        # warm PE p-state over the initial DMA wait; then one agg or proj
        # tile per step so the ACT/DVE epilogue+evict stream stays paced with
        # PE (bursts at phase boundaries starve PE on evict recycling).
        emit_warm(8)
        for i in range(4):
            emit_agg_tile(0, i)
        emit_proj_tile(0, 0)
        emit_proj_tile(0, 1)
        for i in range(4, NT):
            emit_agg_tile(0, i, evict_act=i % 2 == 0)
            emit_proj_tile(0, i - 2)
        for i in range(2):
            emit_agg_tile(1, i, evict_act=i % 2 == 0)
            emit_proj_tile(0, NT - 2 + i)
        for i in range(2, NT):
            emit_agg_tile(1, i, evict_act=i % 2 == 0)
            emit_proj_tile(1, i - 2)
        emit_proj_tile(1, NT - 2, fast_tail=True)
        emit_proj_tile(1, NT - 1, fast_tail=True)

    nc.compile()
    return nc


def _get_nc(has_bf, has_gamma, has_beta):
    key = (has_bf, has_gamma, has_beta)
    if key not in _cache:
        _cache[key] = _build(*key)
    return _cache[key]


def _pack_band(band_mat):
    """band_mat: (N, N) ADJG^T; per-tile groups [diag(i), strip(i-1,i),
    strip(i+1,i)] (strip = the 32 nonzero cols of the off-diag block)."""
    outp = np.zeros((128, BAND_COLS), np.float32)
    for i in range(NT):
        o = _grp_off(i)
        outp[:, o : o + 128] = band_mat[
            128 * i : 128 * (i + 1), 128 * i : 128 * (i + 1)
        ]
        o += 128
        if i > 0:   # left neighbor j=i-1: nonzeros in first 32 cols
            blk = band_mat[128 * (i - 1) : 128 * i, 128 * i : 128 * (i + 1)]
            outp[:, o : o + SW] = blk[:, :SW]
            o += SW
        if i < NT - 1:   # right neighbor j=i+1: nonzeros in last 32 cols
            blk = band_mat[128 * (i + 1) : 128 * (i + 2), 128 * i : 128 * (i + 1)]
            outp[:, o : o + SW] = blk[:, 128 - SW :]
    return outp


def _pack_rows(mat):
    """mat: (D, D') -> [128, KT*D'] with row-tile k at cols [D'*k, D'*(k+1))."""
    Dp = mat.shape[1]
    return np.ascontiguousarray(
        mat.reshape(KT, 128, Dp).transpose(1, 0, 2).reshape(128, KT * Dp)
    )


def prepare_shared(adj_weights, adj_base, node_importance, V_w, semantic_memory,
                   mix_w, mix_b, Wf, bf, gamma, beta):
    """Host-side weight folding -> shared (per-core replicated) device inputs."""
    import ml_dtypes

    bfl = ml_dtypes.bfloat16
    g = 1.0 / (1.0 + np.exp(-node_importance.astype(np.float64)))
    sig = 1.0 / (1.0 + np.exp(-adj_weights.T.astype(np.float64)))
    band_mat = (sig * adj_base.T.astype(np.float64) * g[:, None]).astype(np.float32)
    band = _pack_band(band_mat).astype(bfl)

    WfL_T = Wf[:, :D].T.astype(np.float32)           # (D, D): WfL_T[k, h] = Wf[h, k]
    sm = semantic_memory.astype(np.float32)
    C = sm @ WfL_T                                    # (D, D)
    Delta = (V_w.astype(np.float32).T - sm) @ WfL_T
    q = V_w.astype(np.float32).T @ mix_w.reshape(-1).astype(np.float32)  # (D,)

    shared = {
        "band": band,
        "cD": _pack_rows(C).astype(bfl),
        "dD": _pack_rows(Delta).astype(bfl),
        "qD": np.ascontiguousarray(q.reshape(KT, 128).T).astype(bfl),
        "mb": np.full((128, 1), float(np.asarray(mix_b).reshape(-1)[0]), np.float32),
    }
    has_bf = bool(np.any(bf != 0.0))
    has_gamma = bool(np.any(gamma != 1.0))
    has_beta = bool(np.any(beta != 0.0))
    if has_bf:
        shared["bfb"] = np.ascontiguousarray(np.tile(bf.reshape(1, D), (128, 1)))
    if has_gamma:
        shared["gab"] = np.ascontiguousarray(np.tile(gamma.reshape(1, D), (128, 1)))
    if has_beta:
        shared["beb"] = np.ascontiguousarray(np.tile(beta.reshape(1, D), (128, 1)))
    return shared, (has_bf, has_gamma, has_beta)


def kernel(
    x,
    adj_weights,
    adj_base,
    node_importance,
    V_w,
    semantic_memory,
    mix_w,
    mix_b,
    Wf,
    bf,
    gamma,
    beta,
):
    from concourse.bass_utils import run_bass_kernel_spmd

    import ml_dtypes

    bfl = ml_dtypes.bfloat16

    x = np.asarray(x, np.float32)
    shared, variant = prepare_shared(
        np.asarray(adj_weights, np.float32),
        np.asarray(adj_base, np.float32),
        np.asarray(node_importance, np.float32),
        np.asarray(V_w, np.float32),
        np.asarray(semantic_memory, np.float32),
        np.asarray(mix_w, np.float32),
        np.asarray(mix_b, np.float32),
        np.asarray(Wf, np.float32),
        np.asarray(bf, np.float32),
        np.asarray(gamma, np.float32),
        np.asarray(beta, np.float32),
    )
    nc = _get_nc(*variant)

    in_maps = []
    for c in range(NCORES):
        m = dict(shared)
        m["xN"] = np.ascontiguousarray(x[BL * c : BL * (c + 1)]).astype(bfl)
        in_maps.append(m)

    res = run_bass_kernel_spmd(nc, in_maps, core_ids=list(range(NCORES)))
    return np.concatenate(
        [res.results[c]["out"].astype(np.float32) for c in range(NCORES)], axis=0
    )


# revision 39
# speedup vs baseline: 1.0749x; 1.0066x over previous
"""Trainium2 Bass kernel for nn_AdaptiveCombinatorialComplexLayer.

Math (per batch b):
    adj   = sigmoid(adj_weights) * adj_base          # banded: diagonals {-32,-1,+1,+32}
    xg    = x * sigmoid(node_importance)[None,:,None]
    x_agg = adj @ xg
    v     = x_agg @ V_w.T ; y_pred = x_agg @ sm
    mix   = sigmoid(v @ mix_w.T + mix_b)
    x_proc= mix*v + (1-mix)*y_pred
    out   = LN(x_proc @ Wf[:, :D].T + bf) * gamma + beta

Weight-only folding (host, exact algebra):
    C     = sm @ WfL.T ; Delta = (V_w.T - sm) @ WfL.T    # WfL = Wf[:, :D]
    q     = V_w.T @ mix_w[0]
    BAND[m,n] = sigmoid(adj_weights[n,m]) * adj_base[n,m] * sigmoid(ni)[m]
              (= ADJG^T, the aggregation matrix transposed)

Device pipeline (aggregation FIRST -> one D-wide band matmul, not two):
    uT    = x^T @ BAND            # [feat, node] banded blocks
    aD    = u @ Delta ; aC = u @ C ; aq = u @ q
    mix   = sigmoid(aq + mix_b)
    z     = mix * aD + aC (+ bf)
    out   = LN(z) (* gamma + beta)          # LN stats via bn_stats/bn_aggr

Band structure exploited: for the 32x32 grid, the off-diagonal 128x128 tile
blocks of BAND have nonzeros only in a 32-wide column strip (boundary rows),
so they are packed and matmul'ed as [128, 32] strips.

Sharding: pure data-parallel over batch, 2 batches per core, weights replicated.
"""

import numpy as np

B, N, D, G = 16, 1024, 512, 32
NCORES = 8
BL = B // NCORES          # batches per core
NT = N // 128             # 8 node tiles of 128
KT = D // 128             # 4 feature tiles of 128
LN_EPS = 1e-5
SW = 32                   # off-diagonal strip width

# band packed as per-tile groups [diag(i), strip(i-1,i), strip(i+1,i)] so a
# short DMA prefix unlocks the first aggregation tiles
def _grp_off(i):
    nstrips = 0 if i == 0 else 2 * i - 1
    return 128 * i + SW * nstrips


BAND_COLS = _grp_off(NT - 1) + 128 + SW   # last group: diag + left strip

_cache = {}


def _build(has_bf, has_gamma, has_beta):
    from contextlib import ExitStack

    import concourse.bass as bass
    import concourse.tile as tile
    from concourse import bacc, mybir

    f32 = mybir.dt.float32
    bf16 = mybir.dt.bfloat16
    i32 = mybir.dt.int32
    AF = mybir.ActivationFunctionType
    OP = mybir.AluOpType

    nc = bacc.Bacc(
        "TRN2",
        target_bir_lowering=False,
        debug=False,
        num_devices=NCORES,
    )

    xN = nc.dram_tensor("xN", [BL, N, D], bf16, kind="ExternalInput")
    band = nc.dram_tensor("band", [128, BAND_COLS], bf16, kind="ExternalInput")
    cD = nc.dram_tensor("cD", [128, KT * D], bf16, kind="ExternalInput")
    dD = nc.dram_tensor("dD", [128, KT * D], bf16, kind="ExternalInput")
    qD = nc.dram_tensor("qD", [128, KT], bf16, kind="ExternalInput")
    mb = nc.dram_tensor("mb", [128, 1], f32, kind="ExternalInput")
    if has_bf:
        bfb = nc.dram_tensor("bfb", [128, D], f32, kind="ExternalInput")
    if has_gamma:
        gab = nc.dram_tensor("gab", [128, D], f32, kind="ExternalInput")
    if has_beta:
        beb = nc.dram_tensor("beb", [128, D], f32, kind="ExternalInput")
    out = nc.dram_tensor("out", [BL, N, D], bf16, kind="ExternalOutput")

    def diag_ap(i):
        o = _grp_off(i)
        return band_sb[:, o : o + 128]

    def off_ap(j, i):
        o = _grp_off(i) + 128
        if j > i and i > 0:
            o += SW          # right strip sits after the left strip
        return band_sb[:, o : o + SW]

    with ExitStack() as ctx:
        tc = ctx.enter_context(tile.TileContext(nc))
        const = ctx.enter_context(tc.tile_pool(name="const", bufs=1))

        # ---- persistent SBUF tensors ----
        mb_sb = const.tile([128, 1], f32)
        magic = const.tile([128, 2], i32)     # 0x5f3759df for NR rsqrt
        nc.vector.memset(magic[:], 0x5F3759DF)
        junk = const.tile([128, D], bf16)     # PE p-state warmup operand
        nc.vector.memset(junk[:], 0.0)
        eps_sb = const.tile([128, 1], f32)
        nc.vector.memset(eps_sb[:], LN_EPS)
        band_sb = const.tile([128, BAND_COLS], bf16)
        c_bf = const.tile([128, KT * D], bf16)
        d_bf = const.tile([128, KT * D], bf16)
        q_bf = const.tile([128, KT], bf16)
        if has_bf:
            bf_sb = const.tile([128, D], f32)
            nc.sync.dma_start(bf_sb[:], bfb[:])
        if has_gamma:
            ga_sb = const.tile([128, D], f32)
            nc.sync.dma_start(ga_sb[:], gab[:])
        if has_beta:
            be_sb = const.tile([128, D], f32)
            nc.sync.dma_start(be_sb[:], beb[:])

        xpool = ctx.enter_context(tc.tile_pool(name="xpool", bufs=BL))
        upool = ctx.enter_context(tc.tile_pool(name="upool", bufs=BL))
        xsb = []
        for b in range(BL):
            xsb.append(xpool.tile([128, NT * D], bf16, tag=f"x{b}", name=f"x{b}"))
        usb = [upool.tile([128, NT * D], bf16, tag=f"u{b}", name=f"u{b}")
               for b in range(BL)]

        def load_x(b, jlo, jhi):
            nc.sync.dma_start(
                xsb[b][:, D * jlo : D * jhi].rearrange("p (j d) -> p j d", d=D),
                xN[b, 128 * jlo : 128 * jhi].rearrange("(j p) d -> p j d", p=128),
            )

        # ---- DMA issue order == serial transfer order on the DMA pipe:
        # gate b0 aggregation first, then the weights for the projections,
        # then the rest of x.
        nc.sync.dma_start(band_sb[:], band[:])
        load_x(0, 0, 4)
        nc.sync.dma_start(
            c_bf[:].rearrange("p (k c) -> p k c", k=KT),
            cD[:].rearrange("p (k c) -> p k c", k=KT),
        )
        nc.sync.dma_start(q_bf[:], qD[:])
        nc.sync.dma_start(mb_sb[:], mb[:])
        nc.sync.dma_start(
            d_bf[:].rearrange("p (k c) -> p k c", k=KT),
            dD[:].rearrange("p (k c) -> p k c", k=KT),
        )
        load_x(0, 4, 8)
        load_x(1, 0, 4)
        load_x(1, 4, 8)

        # ---- PSUM pools: 8 banks ----
        psU = ctx.enter_context(tc.tile_pool(name="psU", bufs=2, space="PSUM"))
        psA = ctx.enter_context(tc.tile_pool(name="psA", bufs=2, space="PSUM"))
        psB = ctx.enter_context(tc.tile_pool(name="psB", bufs=2, space="PSUM"))
        psS = ctx.enter_context(tc.tile_pool(name="psS", bufs=2, space="PSUM"))

        epi = ctx.enter_context(tc.tile_pool(name="epi", bufs=4))
        zpool = ctx.enter_context(tc.tile_pool(name="zpool", bufs=4))
        opool = ctx.enter_context(tc.tile_pool(name="opool", bufs=3))

        def emit_warm(n):
            # keep the PE p-state ramp hot across known DMA-pacing stalls;
            # writes are never read (recycled tags)
            for _ in range(n):
                pj = psA.tile([128, D], f32, tag="bigA", name="pj")
                nc.tensor.matmul(pj[:], junk[:, :128], junk[:], start=True, stop=True)

        def emit_agg_tile(b, i, evict_act=True):
            """uT tile i of batch b -> usb[b][:, 512i + 128k] (bf16).

            Off-diagonal neighbor blocks touch only a 32-col strip of the
            output: left neighbor -> cols [0,32), right -> cols [96,128)."""
            pu = psU.tile([128, D], f32, tag="u")
            xl = xsb[b]
            for k in range(KT):
                ks = slice(128 * k, 128 * (k + 1))
                lhs_i = xl[:, D * i + 128 * k : D * i + 128 * (k + 1)]
                base = 128 * k
                # segments of the 128 output cols: (lo, hi, with_off, j_off)
                segs = []
                if i > 0:
                    segs.append((0, SW, True, i - 1))
                    segs.append((SW, 128 if i == NT - 1 else 128 - SW, False, 0))
                else:
                    segs.append((0, 128 - SW, False, 0))
                if i < NT - 1:
                    segs.append((128 - SW, 128, True, i + 1))
                for lo, hi, with_off, joff in segs:
                    osl = pu[:, base + lo : base + hi]
                    nc.tensor.matmul(
                        osl, lhs_i, diag_ap(i)[:, lo:hi],
                        start=True, stop=not with_off,
                    )
                    if with_off:
                        lhs_o = xl[:, D * joff + 128 * k : D * joff + 128 * (k + 1)]
                        nc.tensor.matmul(
                            osl, lhs_o, off_ap(joff, i), start=False, stop=True
                        )
            if evict_act:
                nc.scalar.activation(
                    usb[b][:, D * i : D * (i + 1)], pu[:], AF.Copy
                )
            else:
                nc.vector.tensor_copy(usb[b][:, D * i : D * (i + 1)], pu[:])

        pending_ot = []
        pair_state = {}

        def flush_ot():
            # deferred one tile so the ACT queue never head-blocks on the
            # (late-ready) scale-shift while the next tile's mix is ready
            while pending_ot:
                b, i, z, y, nmr = pending_ot.pop(0)
                ot = opool.tile([128, D], bf16, tag="ot", name="ot")
                nc.scalar.activation(
                    ot[:, : D // 2], z[:, : D // 2], AF.Identity,
                    bias=nmr[:], scale=y[:],
                )
                nc.vector.tensor_scalar(
                    ot[:, D // 2 :], z[:, D // 2 :], y[:], nmr[:],
                    OP.mult, OP.add,
                )
                if has_gamma:
                    nc.vector.tensor_tensor(ot[:], ot[:], ga_sb[:], OP.mult)
                if has_beta:
                    nc.vector.tensor_tensor(ot[:], ot[:], be_sb[:], OP.add)
                nc.sync.dma_start(out[b, 128 * i : 128 * (i + 1), :], ot[:])

        def emit_proj_tile(b, i, fast_tail=False):
            """Project uT tile i through q/C/Delta (q first: its stop releases
            the mix sigmoid early), then the fused epilogue: mix-combine,
            bn LN stats, NR rsqrt, deferred scale-shift + store.

            fast_tail: split the C/Delta projections and the epilogue into
            column halves so the first half's DVE chain overlaps the second
            half's matmuls (used for the last tiles, where no later PE work
            hides the epilogue)."""
            pa_c = psB.tile([128, D], f32, tag="bigB")
            pa_q = psS.tile([128, 1], f32, tag="sm")
            for k in range(KT):
                lhsT = usb[b][:, D * i + 128 * k : D * i + 128 * (k + 1)]
                nc.tensor.matmul(
                    pa_q[:], lhsT, q_bf[:, k : k + 1],
                    start=k == 0, stop=k == KT - 1,
                )
            halves = (
                [(0, D // 2), (D // 2, D)] if fast_tail else [(0, D)]
            )
            # separate PSUM tile per Delta half: a matmul start into a tile
            # that a prior combine read from serializes (tile-granular WAR)
            pa_ds = [psA.tile([128, D], f32, tag="bigA", name="pa_d")
                     for _ in halves]
            for k in range(KT):
                lhsT = usb[b][:, D * i + 128 * k : D * i + 128 * (k + 1)]
                nc.tensor.matmul(
                    pa_c[:], lhsT, c_bf[:, D * k : D * (k + 1)],
                    start=k == 0, stop=k == KT - 1,
                )
            mix = epi.tile([128, 1], f32, tag="mix")
            nc.scalar.activation(
                mix[:], pa_q[:], AF.Sigmoid, bias=mb_sb[:], scale=1.0
            )
            # HW: only one non-scalar PSUM operand per instruction
            csb = epi.tile([128, D], f32, tag="csb")
            nc.scalar.activation(csb[:], pa_c[:], AF.Copy)
            flush_ot()
            z = zpool.tile([128, D], f32, tag="z")
            HD = D // 2
            sums = epi.tile([128, len(halves)], f32, tag="sums")
            for h, (lo, hi) in enumerate(halves):
                pa_d = pa_ds[h]
                for k in range(KT):
                    lhsT = usb[b][:, D * i + 128 * k : D * i + 128 * (k + 1)]
                    nc.tensor.matmul(
                        pa_d[:, : hi - lo], lhsT, d_bf[:, D * k + lo : D * k + hi],
                        start=k == 0, stop=k == KT - 1,
                    )
                nc.vector.scalar_tensor_tensor(
                    z[:, lo:hi], pa_d[:, : hi - lo], mix[:], csb[:, lo:hi],
                    OP.mult, OP.add, accum_out=sums[:, h : h + 1],
                )
                if has_bf:
                    nc.vector.tensor_tensor(
                        z[:, lo:hi], z[:, lo:hi], bf_sb[:, lo:hi], OP.add
                    )
                    nc.vector.tensor_scalar(
                        z[:, lo:hi], z[:, lo:hi], 0.0, None, OP.add,
                        accum_out=sums[:, h : h + 1],
                    )
            # sum of squares: first half on ACT (Square), second on DVE
            sq = epi.tile([128, 2], f32, tag="sq")
            zsq = epi.tile([128, HD], f32, tag="zsq")
            nc.scalar.activation(
                zsq[:], z[:, :HD], AF.Square, accum_out=sq[:, 0:1]
            )
            # mean path depends only on sums: emit before the second sq half
            # so the post-matmul chain is just sq -> va -> NR -> ot
            eng = nc.vector
            mean = epi.tile([128, 1], f32, tag="mean")
            if len(halves) == 1:
                eng.tensor_scalar(mean[:], sums[:, 0:1], 1.0 / D, None, OP.mult)
            else:
                eng.tensor_tensor(
                    sums[:, 0:1], sums[:, 0:1], sums[:, 1:2], OP.add
                )
                eng.tensor_scalar(mean[:], sums[:, 0:1], 1.0 / D, None, OP.mult)
            m2 = epi.tile([128, 1], f32, tag="m2")
            # m2 - eps, so va = sq/D - (m2 - eps) lands in one op
            eng.scalar_tensor_tensor(
                m2[:], mean[:], mean[:], eps_sb[:], OP.mult, OP.subtract
            )
            zsq2 = epi.tile([128, HD], f32, tag="zsq2")
            nc.vector.scalar_tensor_tensor(
                zsq2[:], z[:, HD:], 1.0, z[:, HD:], OP.mult, OP.mult,
                accum_out=sq[:, 1:2],
            )
            eng.tensor_tensor(sq[:, 0:1], sq[:, 0:1], sq[:, 1:2], OP.add)
            va = epi.tile([128, 1], f32, tag="va")
            eng.scalar_tensor_tensor(
                va[:], sq[:, 0:1], 1.0 / D, m2[:], OP.mult, OP.subtract
            )
            ih = epi.tile([128, 1], i32, tag="ih")
            eng.tensor_scalar(ih[:], va[:].bitcast(i32), 1, None, OP.arith_shift_right)
            y = epi.tile([128, 1], f32, tag="y")
            eng.scalar_tensor_tensor(
                y[:].bitcast(i32), magic[:, :1], 0, ih[:], OP.bypass, OP.subtract
            )
            t1 = epi.tile([128, 1], f32, tag="t1")
            # t1 = (y*y)*va in one op (y is scalar-shaped)
            eng.scalar_tensor_tensor(t1[:], y[:], y[:], va[:], OP.mult, OP.mult)
            eng.tensor_scalar(t1[:], t1[:], -0.5, 1.5, OP.mult, OP.add)
            eng.tensor_tensor(y[:], y[:], t1[:], OP.mult)
            nmr = epi.tile([128, 1], f32, tag="nmr")
            eng.tensor_scalar(nmr[:], mean[:], y[:], -1.0, OP.mult, OP.mult)
            if fast_tail:
                # scale-shift halves in parallel on ACT + DVE, store at once
                ot = opool.tile([128, D], bf16, tag="ot", name="ot")
                nc.scalar.activation(
                    ot[:, :HD], z[:, :HD], AF.Identity,
                    bias=nmr[:], scale=y[:],
                )
                nc.vector.tensor_scalar(
                    ot[:, HD:], z[:, HD:], y[:], nmr[:], OP.mult, OP.add
                )
                if has_gamma:
                    nc.vector.tensor_tensor(ot[:], ot[:], ga_sb[:], OP.mult)
                if has_beta:
                    nc.vector.tensor_tensor(ot[:], ot[:], be_sb[:], OP.add)
                nc.sync.dma_start(out[b, 128 * i : 128 * (i + 1), :], ot[:])
            else:
                pending_ot.append((b, i, z, y, nmr))

        # ---- schedule ----
        # warm PE p-state over the initial DMA wait; then one agg or proj
        # tile per step so the ACT/DVE epilogue+evict stream stays paced with
        # PE (bursts at phase boundaries starve PE on evict recycling).
        emit_warm(8)
        for i in range(4):
            emit_agg_tile(0, i)
        emit_proj_tile(0, 0)
        emit_proj_tile(0, 1)
        for i in range(4, NT):
            emit_agg_tile(0, i, evict_act=i % 2 == 0)
            emit_proj_tile(0, i - 2)
        for i in range(2):
            emit_agg_tile(1, i, evict_act=i % 2 == 0)
            emit_proj_tile(0, NT - 2 + i)
        for i in range(2, NT):
            emit_agg_tile(1, i, evict_act=i % 2 == 0)
            emit_proj_tile(1, i - 2)
        emit_proj_tile(1, NT - 2, fast_tail=True)
        emit_proj_tile(1, NT - 1, fast_tail=True)

    nc.compile()
    return nc


def _get_nc(has_bf, has_gamma, has_beta):
    key = (has_bf, has_gamma, has_beta)
    if key not in _cache:
        _cache[key] = _build(*key)
    return _cache[key]


def _pack_band(band_mat):
    """band_mat: (N, N) ADJG^T; per-tile groups [diag(i), strip(i-1,i),
    strip(i+1,i)] (strip = the 32 nonzero cols of the off-diag block)."""
    outp = np.zeros((128, BAND_COLS), np.float32)
    for i in range(NT):
        o = _grp_off(i)
        outp[:, o : o + 128] = band_mat[
            128 * i : 128 * (i + 1), 128 * i : 128 * (i + 1)
        ]
        o += 128
        if i > 0:   # left neighbor j=i-1: nonzeros in first 32 cols
            blk = band_mat[128 * (i - 1) : 128 * i, 128 * i : 128 * (i + 1)]
            outp[:, o : o + SW] = blk[:, :SW]
            o += SW
        if i < NT - 1:   # right neighbor j=i+1: nonzeros in last 32 cols
            blk = band_mat[128 * (i + 1) : 128 * (i + 2), 128 * i : 128 * (i + 1)]
            outp[:, o : o + SW] = blk[:, 128 - SW :]
    return outp


def _pack_rows(mat):
    """mat: (D, D') -> [128, KT*D'] with row-tile k at cols [D'*k, D'*(k+1))."""
    Dp = mat.shape[1]
    return np.ascontiguousarray(
        mat.reshape(KT, 128, Dp).transpose(1, 0, 2).reshape(128, KT * Dp)
    )


def prepare_shared(adj_weights, adj_base, node_importance, V_w, semantic_memory,
                   mix_w, mix_b, Wf, bf, gamma, beta):
    """Host-side weight folding -> shared (per-core replicated) device inputs."""
    import ml_dtypes

    bfl = ml_dtypes.bfloat16
    g = 1.0 / (1.0 + np.exp(-node_importance.astype(np.float64)))
    sig = 1.0 / (1.0 + np.exp(-adj_weights.T.astype(np.float64)))
    band_mat = (sig * adj_base.T.astype(np.float64) * g[:, None]).astype(np.float32)
    band = _pack_band(band_mat).astype(bfl)

    WfL_T = Wf[:, :D].T.astype(np.float32)           # (D, D): WfL_T[k, h] = Wf[h, k]
    sm = semantic_memory.astype(np.float32)
    C = sm @ WfL_T                                    # (D, D)
    Delta = (V_w.astype(np.float32).T - sm) @ WfL_T
    q = V_w.astype(np.float32).T @ mix_w.reshape(-1).astype(np.float32)  # (D,)

    shared = {
        "band": band,
        "cD": _pack_rows(C).astype(bfl),
        "dD": _pack_rows(Delta).astype(bfl),
        "qD": np.ascontiguousarray(q.reshape(KT, 128).T).astype(bfl),
        "mb": np.full((128, 1), float(np.asarray(mix_b).reshape(-1)[0]), np.float32),
    }
    has_bf = bool(np.any(bf != 0.0))
    has_gamma = bool(np.any(gamma != 1.0))
    has_beta = bool(np.any(beta != 0.0))
    if has_bf:
        shared["bfb"] = np.ascontiguousarray(np.tile(bf.reshape(1, D), (128, 1)))
    if has_gamma:
        shared["gab"] = np.ascontiguousarray(np.tile(gamma.reshape(1, D), (128, 1)))
    if has_beta:
        shared["beb"] = np.ascontiguousarray(np.tile(beta.reshape(1, D), (128, 1)))
    return shared, (has_bf, has_gamma, has_beta)


def kernel(
    x,
    adj_weights,
    adj_base,
    node_importance,
    V_w,
    semantic_memory,
    mix_w,
    mix_b,
    Wf,
    bf,
    gamma,
    beta,
):
    from concourse.bass_utils import run_bass_kernel_spmd

    import ml_dtypes

    bfl = ml_dtypes.bfloat16

    x = np.asarray(x, np.float32)
    shared, variant = prepare_shared(
        np.asarray(adj_weights, np.float32),
        np.asarray(adj_base, np.float32),
        np.asarray(node_importance, np.float32),
        np.asarray(V_w, np.float32),
        np.asarray(semantic_memory, np.float32),
        np.asarray(mix_w, np.float32),
        np.asarray(mix_b, np.float32),
        np.asarray(Wf, np.float32),
        np.asarray(bf, np.float32),
        np.asarray(gamma, np.float32),
        np.asarray(beta, np.float32),
    )
    nc = _get_nc(*variant)

    in_maps = []
    for c in range(NCORES):
        m = dict(shared)
        m["xN"] = np.ascontiguousarray(x[BL * c : BL * (c + 1)]).astype(bfl)
        in_maps.append(m)

    res = run_bass_kernel_spmd(nc, in_maps, core_ids=list(range(NCORES)))
    return np.concatenate(
        [res.results[c]["out"].astype(np.float32) for c in range(NCORES)], axis=0
    )
